# revision 14
# baseline (speedup 1.0000x reference)
"""TRN2 Bass kernel for nn_BlendEmoBackbone: gated audio mixer + low-rank
multiplicative fusion, data-parallel over batch on 8 NeuronCores.

Strategy (v2, bf16):
- Pure data parallel: each core handles B/8 = 512 batch rows; the gate MLP
  weights and the LMF factor tensors are replicated.
- All activations kept in transposed [feature, batch] layout on-chip so every
  matmul contracts over the partition dim; tokens are transposed on the host.
- Every matmul operand is bf16 (full PE rate + FWL weight loads + halved
  HBM traffic for the 168MB factor tensor); PSUM accumulation stays f32.
- LayerNorms over the feature dim use PE ones-matmuls for partition sums,
  with the -mu term folded into gate matmuls as an extra contraction row.
- The LMF where(mask, z, 1) and the x_aug ones-column are folded into the
  factor matmul as a K=2 tail tile ([bias_row; ones_row] x [mask; 1-mask]).
- rank_w is folded into the audio factor slices on the host.
- LMF rank accumulation runs on PE via an identity-matmul into a PSUM bank.
- The audio token squares and the s-part of the gate-MLP layer-1 matmul are
  shared across the three "other" gates.
"""

import numpy as np
import ml_dtypes
from contextlib import ExitStack

import concourse.bass as bass
from concourse import bacc
import concourse.tile as tile
from concourse import mybir
from concourse.bass_utils import run_bass_kernel_spmd

B, M, H, R = 4096, 4, 1024, 10
NCORES = 8
BS = B // NCORES          # 512 batch rows per core
MID = 512
P = 128
HT = H // P               # 8 h-tiles
MT = MID // P             # 4 mid-tiles
D3 = 3 * H
OTHERS = (0, 2, 3)
AUDIO = 1
EPS = 1e-5

f32 = mybir.dt.float32
bf16 = mybir.dt.bfloat16
nbf = ml_dtypes.bfloat16
AF = mybir.ActivationFunctionType
OP = mybir.AluOpType

TRACE = False
LAST_RESULTS = None

_cached = {}

# Pattern-group ordering (pattern code bit m = modality m active), chosen so
# each modality's active columns form few contiguous runs; ends at 1111 so
# the overflow segment merges with every modality's last run.
PORDER = [0, 4, 5, 13, 9, 1, 11, 3, 7, 6, 2, 10, 8, 12, 14, 15]


def _build(spans):
    nc = bacc.Bacc("TRN2", target_bir_lowering=False, debug=False)

    # ---- DRAM parameters (per core) ----
    tokT = nc.declare_dram_parameter("tokT", [M, H, BS], bf16, isOutput=False)
    # bf16 mask rows: 0-2 pv_j, 3-5 mo_j, 6 am(aum), 7 ma
    mrows = nc.declare_dram_parameter("mrows", [8, BS], bf16, isOutput=False)
    cmrows = nc.declare_dram_parameter("cmrows", [3, BS], bf16, isOutput=False)
    uv = nc.declare_dram_parameter("uv", [M, 2, BS], bf16, isOutput=False)
    WGO = nc.declare_dram_parameter("WGO", [3 * HT, P, MID], bf16, isOutput=False)
    WGA = nc.declare_dram_parameter("WGA", [3 * HT, P, MID], bf16, isOutput=False)
    WGOe = nc.declare_dram_parameter("WGOe", [1, MID], bf16, isOutput=False)
    WGAe = nc.declare_dram_parameter("WGAe", [1, MID], bf16, isOutput=False)
    W2 = nc.declare_dram_parameter("W2", [P, MT, 2], bf16, isOutput=False)
    CB = nc.declare_dram_parameter("CB", [P, 8], f32, isOutput=False)
    SC = nc.declare_dram_parameter("SC", [1, 8], f32, isOutput=False)
    # [ho, p, kt, c]: wT[kt*P+p, ho*P+c] — one contiguous [P, HT*P] row per ho
    A2OT = nc.declare_dram_parameter("A2OT", [HT, P, HT, P], bf16, isOutput=False)
    O2AT = nc.declare_dram_parameter("O2AT", [HT, P, HT, P], bf16, isOutput=False)
    OUTWT = nc.declare_dram_parameter("OUTWT", [HT, P, HT, P], bf16, isOutput=False)
    # cols: ln_o_w 0:8, ln_o_b 8:16, ln_a_w 16:24, ln_a_b 24:32,
    #       ln1w 32:40, ln1b 40:48, ln2w 48:56, ln2b 56:64, outb 64:72, lmfb 72:80
    LNV = nc.declare_dram_parameter("LNV", [P, 80], f32, isOutput=False)
    # FT[ht, r, m, p, kt, c]: main factor blocks for kt<8; [p,8,:] rows 0/1
    # hold [bias_row; ones_or_rankw_row] for the K=2 tail matmul.
    FT = nc.declare_dram_parameter("FT", [HT, R, P, M, 9, P], bf16, isOutput=False)
    EYE = nc.declare_dram_parameter("EYE", [P, P], bf16, isOutput=False)
    OUT = nc.declare_dram_parameter("outT", [H, BS], f32, isOutput=True)

    with tile.TileContext(nc) as tc, ExitStack() as ctx:
        kp = ctx.enter_context(tc.tile_pool(name="konst", bufs=1))
        tokp = ctx.enter_context(tc.tile_pool(name="tokp", bufs=1))
        big = ctx.enter_context(tc.tile_pool(name="big", bufs=1))
        wk = ctx.enter_context(tc.tile_pool(name="wk", bufs=2))
        bcp = ctx.enter_context(tc.tile_pool(name="bcp", bufs=1))
        sqp = ctx.enter_context(tc.tile_pool(name="sqp", bufs=2))
        wgp = ctx.enter_context(tc.tile_pool(name="wgp", bufs=2))
        ftp = ctx.enter_context(tc.tile_pool(name="ftp", bufs=4))
        rowp = ctx.enter_context(tc.tile_pool(name="rowp", bufs=1))
        ppz = ctx.enter_context(tc.tile_pool(name="ppz", bufs=5, space="PSUM"))
        pps = ctx.enter_context(tc.tile_pool(name="pps", bufs=1, space="PSUM"))
        ppo = ctx.enter_context(tc.tile_pool(name="ppo", bufs=1, space="PSUM"))
        ppa = ctx.enter_context(tc.tile_pool(name="ppa", bufs=1, space="PSUM"))

        # ---- constants / small loads ----
        ones_k = kp.tile([P, 1], bf16)
        nc.vector.memset(ones_k, 1.0)
        ones1 = kp.tile([1, P], bf16)
        nc.vector.memset(ones1, 1.0)
        eye = kp.tile([P, P], bf16)
        nc.sync.dma_start(out=eye, in_=EYE.ap())

        def bc_row_dma(dst, src_ap):
            nc.sync.dma_start(
                out=dst,
                in_=bass.AP(
                    tensor=src_ap.tensor, offset=src_ap.offset, ap=[[0, P], [1, BS]]
                ),
            )

        mt_ = []
        for i in range(8):
            t = kp.tile([P, BS], bf16, tag=f"mk_{i}")
            bc_row_dma(t, mrows.ap()[i : i + 1, :])
            mt_.append(t)
        pv_t, mo_t, am_t, ma_t = mt_[0:3], mt_[3:6], mt_[6], mt_[7]
        cm_t = []
        for i in range(3):
            t = kp.tile([P, BS], bf16, tag=f"cm_{i}")
            bc_row_dma(t, cmrows.ap()[i : i + 1, :])
            cm_t.append(t)
        uvt = []
        for m in range(M):
            t = kp.tile([2, BS], bf16, tag=f"uv_{m}")
            nc.sync.dma_start(out=t, in_=uv.ap()[m])
            uvt.append(t)
        cbt = kp.tile([P, 8], f32)
        nc.sync.dma_start(out=cbt, in_=CB.ap())
        sct = kp.tile([1, 8], f32)
        nc.sync.dma_start(out=sct, in_=SC.ap())
        lnv = kp.tile([P, 80], f32)
        nc.sync.dma_start(out=lnv, in_=LNV.ap())
        w2t = kp.tile([P, MT, 2], bf16)
        nc.sync.dma_start(out=w2t, in_=W2.ap())

        # ---- tokens (transposed) ----
        tok = tokp.tile([P, M, HT, BS], bf16)
        for m in range(M):
            src = tokT.ap()[m].rearrange("(ht p) b -> p ht b", p=P)
            for ht in range(HT):
                nc.sync.dma_start(out=tok[:, m, ht, :], in_=src[:, ht, :])

        def tk(m, kt):
            return tok[:, m, kt, :]

        # ---- helpers ----
        def pe_warm(row, tag):
            """Tiny matmul that depends on a mid-chain row: keeps the PE HAM
            activity window busy during serial LN chains so the clock gate
            stays at 2.4 GHz. ~60 cycles each."""
            dt = ppa.tile([1, 1], f32, tag="acc", name=f"warm_{tag}")
            nc.tensor.matmul(dt, ones_k[0:1, 0:1], row, start=True, stop=True)

        def ln_rows(stat, n, tag):
            """From psum stat banks (sum, sumsq) compute negmu [1,BS] bf16
            and rinv [1,BS] bf16 rows."""
            statA, statB = stat
            negmu = rowp.tile([1, BS], bf16, tag="negmu", name=f"negmu_{tag}")
            nc.scalar.activation(negmu, statA, AF.Copy, bias=0.0, scale=-1.0 / n)
            ex2 = rowp.tile([1, BS], f32, tag="ex2", name=f"ex2_{tag}")
            nc.scalar.activation(ex2, statB, AF.Copy, bias=0.0, scale=1.0 / n)
            msq = rowp.tile([1, BS], f32, tag="msq", name=f"msq_{tag}")
            nc.scalar.activation(msq, negmu, AF.Square)
            pe_warm(negmu[0:1, 0:1], f"a_{tag}")
            nc.vector.tensor_sub(ex2, ex2, msq)                      # var in place
            rinv = rowp.tile([1, BS], bf16, tag="rinv", name=f"rinv_{tag}")
            # rsqrt(v+eps) via abs_reciprocal_sqrt (v+eps > 0 always;
            # plain Rsqrt is guard-blocked in bass).
            nc.scalar.activation(
                rinv, ex2, AF.Abs_reciprocal_sqrt, bias=sct[0:1, 2:3], scale=1.0
            )
            pe_warm(rinv[0:1, 0:1], f"b_{tag}")
            return negmu, rinv

        def bcast(row, tag):
            """Broadcast a [1,BS] bf16 row to a [P,BS] bf16 sbuf tile."""
            po = ppo.tile([P, BS], f32, tag="outer")
            nc.tensor.matmul(po, ones1, row, start=True, stop=True)
            sb = bcp.tile([P, BS], bf16, tag=f"bc_{tag}")
            nc.scalar.activation(sb, po, AF.Copy, bias=0.0, scale=1.0)
            return sb

        def colsum_stats(stat, pairs):
            """Accumulate sum (bank A) and sumsq (bank B) over the given
            (tile, square_tile) pairs of [P,BS] bf16 tiles."""
            statA, statB = stat
            n = len(pairs)
            for i, (t, sq) in enumerate(pairs):
                nc.tensor.matmul(statA, ones_k, t, start=(i == 0), stop=(i == n - 1))
                nc.tensor.matmul(statB, ones_k, sq, start=(i == 0), stop=(i == n - 1))

        # ---- a2o = audio @ a2o_w.T, in T layout [H, BS] ----
        a2or = big.tile([P, HT, BS], bf16, tag="axr")
        for ho in range(HT):
            wt = wgp.tile([P, HT, P], bf16, tag="ww")
            nc.sync.dma_start(out=wt, in_=A2OT.ap()[ho])
            ps = ppz.tile([P, BS], f32, tag="z")
            for kt in range(HT):
                nc.tensor.matmul(ps, wt[:, kt, :], tk(AUDIO, kt), start=(kt == 0), stop=(kt == HT - 1))
            nc.scalar.activation(a2or[:, ho, :], ps, AF.Copy, bias=0.0, scale=1.0)

        # ---- audio squares, shared by gates 0-2 (s) and gate 3 (t) ----
        asq = big.tile([P, HT, BS], bf16, tag="asq")
        for kt in range(HT):
            nc.scalar.activation(asq[:, kt, :], tk(AUDIO, kt), AF.Square)

        # ---- s-part of gate_other layer 1, shared by gates 0-2 ----
        spart = big.tile([P, MT, BS], bf16, tag="spart")
        sps = [ppz.tile([P, BS], f32, tag="z", name=f"sps{mt}") for mt in range(MT)]
        for kt in range(HT):
            wt = wgp.tile([P, MID], bf16, tag="wg")
            nc.sync.dma_start(out=wt, in_=WGO.ap()[HT + kt])
            for mt in range(MT):
                nc.tensor.matmul(
                    sps[mt], wt[:, mt * P : (mt + 1) * P], tk(AUDIO, kt),
                    start=(kt == 0), stop=(kt == HT - 1),
                )
        for mt in range(MT):
            nc.scalar.activation(spart[:, mt, :], sps[mt], AF.Copy, bias=0.0, scale=1.0)

        omt = big.tile([P, HT, BS], bf16, tag="om")  # others_mean accumulator
        mix_src = {"x": a2or}  # a2o for others-gates, o2a for the audio gate

        def gate_and_mix(j, mj):
            """j: 0..2 index into OTHERS, or 3 for the audio gate."""
            is_audio = j == 3
            t_m = AUDIO if is_audio else mj

            def s_tile(kt):
                return omt[:, kt, :] if is_audio else tk(AUDIO, kt)

            # |t-s| tiles + squares + LN stats over the 3H concat features
            _st = pps.tile([33, BS], f32, tag="st", name="statg")
            stat = (_st[0:1, :], _st[32:33, :])
            abs_t = big.tile([P, HT, BS], bf16, tag="abs")
            pairs = []
            for kt in range(HT):
                d = wk.tile([P, BS], bf16, tag="d")
                nc.vector.tensor_sub(d, tk(t_m, kt), s_tile(kt))
                nc.scalar.activation(abs_t[:, kt, :], d, AF.Abs)
                sqd = sqp.tile([P, BS], bf16, tag="sq")
                nc.scalar.activation(sqd, d, AF.Square)
                pairs.append((abs_t[:, kt, :], sqd))
                if is_audio:
                    pairs.append((tk(t_m, kt), asq[:, kt, :]))
                    sqs = sqp.tile([P, BS], bf16, tag="sq")
                    nc.scalar.activation(sqs, s_tile(kt), AF.Square)
                    pairs.append((s_tile(kt), sqs))
                else:
                    sqt = sqp.tile([P, BS], bf16, tag="sq")
                    nc.scalar.activation(sqt, tk(t_m, kt), AF.Square)
                    pairs.append((tk(t_m, kt), sqt))
                    pairs.append((s_tile(kt), asq[:, kt, :]))
            colsum_stats(stat, pairs)
            negmu, rinv = ln_rows(stat, D3, "g")

            # gate layer 1: psum[mt] = W~.T @ [t; s; |t-s|] - mu*c1
            # (s-part pre-computed in `spart` for the non-audio gates)
            WG = WGA if is_audio else WGO
            wge = rowp.tile([1, MID], bf16, tag="wge", name=f"wge{j}")
            nc.sync.dma_start(out=wge, in_=(WGAe if is_audio else WGOe).ap())
            gps = [ppz.tile([P, BS], f32, tag="z", name=f"gps{mt}") for mt in range(MT)]
            kts = range(3 * HT) if is_audio else [k for k in range(3 * HT) if k // HT != 1]
            first = True
            for kt in kts:
                wt = wgp.tile([P, MID], bf16, tag="wg")
                nc.sync.dma_start(out=wt, in_=WG.ap()[kt])
                part, k = kt // HT, kt % HT
                rhs = tk(t_m, k) if part == 0 else (s_tile(k) if part == 1 else abs_t[:, k, :])
                for mt in range(MT):
                    nc.tensor.matmul(
                        gps[mt], wt[:, mt * P : (mt + 1) * P], rhs,
                        start=first, stop=False,
                    )
                first = False
            for mt in range(MT):
                nc.tensor.matmul(
                    gps[mt], wge[0:1, mt * P : (mt + 1) * P], negmu,
                    start=False, stop=True,
                )
            rb = bcast(rinv, "rb")
            cb_off = 4 if is_audio else 0
            col = 1 if is_audio else 0
            _gp = pps.tile([33, BS], f32, tag="st", name="gp")
            gp = _gp[0:1, :]
            for mt in range(MT):
                hm = wk.tile([P, BS], bf16, tag="hm")
                if is_audio:
                    nc.vector.tensor_mul(hm, gps[mt], rb)
                else:
                    nc.vector.tensor_add(hm, gps[mt], spart[:, mt, :])
                    nc.vector.tensor_mul(hm, hm, rb)
                hg1 = wk.tile([P, BS], bf16, tag="hg", name=f"hg{mt}")
                nc.scalar.activation(
                    hg1, hm, AF.Gelu,
                    bias=cbt[:, cb_off + mt : cb_off + mt + 1], scale=1.0,
                )
                nc.tensor.matmul(
                    gp, w2t[:, mt, col : col + 1], hg1,
                    start=(mt == 0), stop=(mt == MT - 1),
                )
            g_row = rowp.tile([1, BS], bf16, tag="g_row")
            nc.scalar.activation(
                g_row, gp, AF.Sigmoid,
                bias=sct[0:1, col : col + 1], scale=1.0,
            )
            pe_warm(g_row[0:1, 0:1], f"g{j}")
            gb = bcast(g_row, "gb")

            # pre = t + g * (a2o | o2a); LN over H; blend into tok in place
            src = mix_src["x"]
            pre = big.tile([P, HT, BS], bf16, tag="abs", name="pre")
            _st2 = pps.tile([33, BS], f32, tag="st", name="stat2")
            stat2 = (_st2[0:1, :], _st2[32:33, :])
            pairs2 = []
            for kt in range(HT):
                tmp = wk.tile([P, BS], bf16, tag="hm")
                nc.vector.tensor_mul(tmp, gb, src[:, kt, :])
                nc.vector.tensor_add(pre[:, kt, :], tmp, tk(t_m, kt))
                sq = sqp.tile([P, BS], bf16, tag="sq")
                nc.scalar.activation(sq, pre[:, kt, :], AF.Square)
                pairs2.append((pre[:, kt, :], sq))
            colsum_stats(stat2, pairs2)
            negmu2, rinv2 = ln_rows(stat2, H, "u")
            mb = bcast(negmu2, "mb")
            rb2 = bcast(rinv2, "rb2")
            wcol = 16 if is_audio else 0
            bcol = 24 if is_audio else 8
            for kt in range(HT):
                nc.vector.tensor_add(pre[:, kt, :], pre[:, kt, :], mb)
                nc.vector.tensor_mul(pre[:, kt, :], pre[:, kt, :], rb2)
                nc.vector.tensor_scalar(
                    pre[:, kt, :], pre[:, kt, :],
                    lnv[:, wcol + kt : wcol + kt + 1], lnv[:, bcol + kt : bcol + kt + 1],
                    op0=OP.mult, op1=OP.add,
                )
                # blend = big_mask*t + small_mask*(upd - t), in place into tok
                bm = ma_t if is_audio else mo_t[j]
                sm = am_t if is_audio else pv_t[j]
                d2 = wk.tile([P, BS], bf16, tag="d", name="d2")
                if kt % 2 == 0:
                    pe_warm(pre[0:1, kt, 0:1], f"bl{j}_{kt}")
                nc.vector.tensor_sub(d2, pre[:, kt, :], tk(t_m, kt))
                nc.vector.tensor_mul(d2, d2, sm)
                nc.vector.tensor_mul(tk(t_m, kt), tk(t_m, kt), bm)
                nc.vector.tensor_add(tk(t_m, kt), tk(t_m, kt), d2)
                if not is_audio:
                    if j == 0:
                        nc.vector.tensor_mul(omt[:, kt, :], cm_t[j], tk(mj, kt))
                    else:
                        tmp2 = wk.tile([P, BS], bf16, tag="hm")
                        nc.vector.tensor_mul(tmp2, cm_t[j], tk(mj, kt))
                        nc.vector.tensor_add(omt[:, kt, :], omt[:, kt, :], tmp2)

        for j, mj in enumerate(OTHERS):
            gate_and_mix(j, mj)

        # ---- o2a = others_mean @ o2a_w.T ----
        o2ar = big.tile([P, HT, BS], bf16, tag="axr")
        for ho in range(HT):
            wt = wgp.tile([P, HT, P], bf16, tag="ww")
            nc.sync.dma_start(out=wt, in_=O2AT.ap()[ho])
            ps = ppz.tile([P, BS], f32, tag="z")
            for kt in range(HT):
                nc.tensor.matmul(ps, wt[:, kt, :], omt[:, kt, :], start=(kt == 0), stop=(kt == HT - 1))
            nc.scalar.activation(o2ar[:, ho, :], ps, AF.Copy, bias=0.0, scale=1.0)
        mix_src["x"] = o2ar

        gate_and_mix(3, AUDIO)

        # ---- LMF: z per (ht,r,m); prod over m on DVE; rank-sum on PE ----
        accs = big.tile([P, HT, BS], bf16, tag="accs")
        for ht in range(HT):
            acc_ps = ppa.tile([P, BS], f32, tag="acc", name=f"acc{ht}")
            for r in range(R):
                zps = []
                s01 = None
                z0c = None
                ftm = ftp.tile([P, M, 9, P], bf16, tag="ft")
                nc.sync.dma_start(out=ftm, in_=FT.ap()[ht, r])
                for m in range(M):
                    zp = ppz.tile([P, BS], f32, tag="z")
                    # tail first (full width): bias*mask + 1*(1-mask); main
                    # matmuls then accumulate only over active column spans.
                    nc.tensor.matmul(zp, ftm[0:2, m, 8, :], uvt[m], start=True, stop=False)
                    sp = spans[m]
                    for kt in range(HT):
                        for si, (a, b) in enumerate(sp):
                            inst = nc.tensor.matmul(
                                zp[:, a:b], ftm[:, m, kt, :], tok[:, m, kt, a:b],
                                start=False,
                                stop=(kt == HT - 1 and si == len(sp) - 1),
                            )
                            if si > 0:
                                # same stationary as the previous span matmul —
                                # skip the redundant LDWEIGHTS
                                inst.ins.ldweights = False
                    zps.append(zp)
                    # PSUM has one DVE read port: stage one factor of each
                    # pair through SBUF via the (otherwise idle) ScalarE.
                    if m == 0:
                        z0c = wk.tile([P, BS], bf16, tag="s01", name="z0c")
                        nc.scalar.activation(z0c, zps[0], AF.Copy, bias=0.0, scale=1.0)
                    if m == 1:
                        s01 = wk.tile([P, BS], bf16, tag="s01", name="s01")
                        nc.vector.tensor_mul(s01, z0c, zps[1])
                    if m == 2:
                        z0c = wk.tile([P, BS], bf16, tag="s23", name="z2c")
                        nc.scalar.activation(z0c, zps[2], AF.Copy, bias=0.0, scale=1.0)
                s23 = wk.tile([P, BS], bf16, tag="s23", name="s23")
                nc.vector.tensor_mul(s23, z0c, zps[3])
                prod = wk.tile([P, BS], bf16, tag="prod")
                nc.vector.tensor_mul(prod, s01, s23)
                nc.tensor.matmul(acc_ps, eye, prod, start=(r == 0), stop=(r == R - 1))
            # accs = acc + lmf_bias (bias col folded into the copy)
            nc.scalar.activation(
                accs[:, ht, :], acc_ps, AF.Identity,
                bias=lnv[:, 72 + ht : 72 + ht + 1], scale=1.0,
            )

        # ---- output MLP ----
        _st3 = pps.tile([33, BS], f32, tag="st", name="stat3")
        stat3 = (_st3[0:1, :], _st3[32:33, :])
        pairs3 = []
        for kt in range(HT):
            sq = sqp.tile([P, BS], bf16, tag="sq")
            nc.scalar.activation(sq, accs[:, kt, :], AF.Square)
            pairs3.append((accs[:, kt, :], sq))
        colsum_stats(stat3, pairs3)
        negmu3, rinv3 = ln_rows(stat3, H, "l1")
        mb3 = bcast(negmu3, "mb")
        rb3 = bcast(rinv3, "rb2")
        for kt in range(HT):
            nc.vector.tensor_add(accs[:, kt, :], accs[:, kt, :], mb3)
            nc.vector.tensor_mul(accs[:, kt, :], accs[:, kt, :], rb3)
            nc.vector.tensor_scalar(
                accs[:, kt, :], accs[:, kt, :],
                lnv[:, 32 + kt : 32 + kt + 1], lnv[:, 40 + kt : 40 + kt + 1],
                op0=OP.mult, op1=OP.add,
            )

        # h2 = gelu(h1 @ out_w.T + out_b); LN2; write out
        h2 = big.tile([P, HT, BS], bf16, tag="abs")
        _st4 = pps.tile([33, BS], f32, tag="st", name="stat4")
        stat4 = (_st4[0:1, :], _st4[32:33, :])
        pairs4 = []
        for ho in range(HT):
            wt = wgp.tile([P, HT, P], bf16, tag="ww")
            nc.sync.dma_start(out=wt, in_=OUTWT.ap()[ho])
            ps = ppz.tile([P, BS], f32, tag="z")
            for kt in range(HT):
                nc.tensor.matmul(ps, wt[:, kt, :], accs[:, kt, :], start=(kt == 0), stop=(kt == HT - 1))
            nc.scalar.activation(
                h2[:, ho, :], ps, AF.Gelu, bias=lnv[:, 64 + ho : 64 + ho + 1], scale=1.0
            )
            sq = sqp.tile([P, BS], bf16, tag="sq")
            nc.scalar.activation(sq, h2[:, ho, :], AF.Square)
            pairs4.append((h2[:, ho, :], sq))
        colsum_stats(stat4, pairs4)
        negmu4, rinv4 = ln_rows(stat4, H, "l2")
        mb4 = bcast(negmu4, "mb")
        rb4 = bcast(rinv4, "rb2")
        for kt in range(HT):
            fin = wk.tile([P, BS], f32, tag="fin")
            nc.vector.tensor_add(fin, h2[:, kt, :], mb4)
            nc.vector.tensor_mul(fin, fin, rb4)
            nc.vector.tensor_scalar(
                fin, fin, lnv[:, 48 + kt : 48 + kt + 1], lnv[:, 56 + kt : 56 + kt + 1],
                op0=OP.mult, op1=OP.add,
            )
            nc.sync.dma_start(out=OUT.ap()[kt * P : (kt + 1) * P, :], in_=fin)

    nc.compile()
    return nc


def _host_prep(inputs, perms):
    tokens = np.asarray(inputs["tokens"], np.float32)
    token_mask = np.asarray(inputs["token_mask"])
    mask_f = token_mask.astype(np.float32)

    mo = mask_f[:, list(OTHERS)]                      # [B,3]
    ma = mask_f[:, AUDIO]                             # [B]
    pv = mo * ma[:, None]                             # [B,3]
    winv = (1.0 / np.clip(mo.sum(1), 1.0, None)).astype(np.float32)
    aum = ma * (mo.max(1) > 0)                        # [B]

    go_w1 = np.asarray(inputs["go_w1"], np.float32)
    ga_w1 = np.asarray(inputs["ga_w1"], np.float32)

    def gate_prep(w1, b1, lnw, lnb):
        W1w = w1 * lnw[None, :]                       # [MID, 3H]
        c1 = np.ascontiguousarray(W1w.sum(1).reshape(1, MID))
        cb = w1 @ lnb + b1                            # [MID]
        Wblocks = np.ascontiguousarray(W1w.T).reshape(3 * HT, P, MID)
        return Wblocks, c1, cb

    WGOv, c1go, cbgo = gate_prep(
        go_w1, np.asarray(inputs["go_b1"], np.float32),
        np.asarray(inputs["ln_go_w"], np.float32), np.asarray(inputs["ln_go_b"], np.float32),
    )
    WGAv, c1ga, cbga = gate_prep(
        ga_w1, np.asarray(inputs["ga_b1"], np.float32),
        np.asarray(inputs["ln_ga_w"], np.float32), np.asarray(inputs["ln_ga_b"], np.float32),
    )
    CBv = np.ascontiguousarray(
        np.concatenate([cbgo.reshape(MT, P).T, cbga.reshape(MT, P).T], axis=1)
    ).astype(np.float32)                              # [P, 8]
    W2v = np.stack(
        [np.asarray(inputs["go_w2"], np.float32).reshape(MID),
         np.asarray(inputs["ga_w2"], np.float32).reshape(MID)], axis=1
    )                                                 # [MID, 2]
    W2v = np.ascontiguousarray(W2v.reshape(MT, P, 2).transpose(1, 0, 2))
    SCv = np.zeros((1, 8), np.float32)
    SCv[0, 0] = np.asarray(inputs["go_b2"], np.float32).reshape(-1)[0]
    SCv[0, 1] = np.asarray(inputs["ga_b2"], np.float32).reshape(-1)[0]
    SCv[0, 2] = EPS

    def tile_blocks(w):
        wt = np.ascontiguousarray(np.asarray(w, np.float32).T)    # [H_in, H_out]
        # [ho, p, kt, c] = wT[kt*P+p, ho*P+c]
        return np.ascontiguousarray(
            wt.reshape(HT, P, HT, P).transpose(2, 1, 0, 3)
        ).astype(nbf)

    A2OTv = tile_blocks(inputs["a2o_w"])
    O2ATv = tile_blocks(inputs["o2a_w"])
    OUTWTv = tile_blocks(inputs["out_w"])

    def cols(name):
        return np.asarray(inputs[name], np.float32).reshape(HT, P).T

    LNVv = np.zeros((P, 80), np.float32)
    for i, name in enumerate(
        ["ln_o_w", "ln_o_b", "ln_a_w", "ln_a_b", "out_ln1_w", "out_ln1_b",
         "out_ln2_w", "out_ln2_b", "out_b", "lmf_bias"]
    ):
        LNVv[:, 8 * i : 8 * (i + 1)] = cols(name)

    factors = np.asarray(inputs["factors"], np.float32)
    rank_w = np.asarray(inputs["rank_w"], np.float32)
    Ff = factors.copy()
    Ff[AUDIO] = Ff[AUDIO] * rank_w[:, None, None]
    FTv = np.zeros((HT, R, P, M, 9, P), np.float32)
    # main [m, r, kt, pk, ht, ph] -> [ht, r, pk, m, kt, ph]
    main = Ff[:, :, 1:, :].reshape(M, R, HT, P, HT, P)
    FTv[:, :, :, :, :8, :] = main.transpose(4, 1, 3, 0, 2, 5)
    bias = Ff[:, :, 0, :].reshape(M, R, HT, P)           # [m, r, ht, ph]
    FTv[:, :, 0, :, 8, :] = bias.transpose(2, 1, 0, 3)
    ones_row = np.ones((HT, R, M, P), np.float32)
    ones_row[:, :, AUDIO, :] = rank_w[None, :, None]
    FTv[:, :, 1, :, 8, :] = ones_row
    FTv = FTv.astype(nbf)

    shared = dict(
        WGO=WGOv.astype(nbf), WGA=WGAv.astype(nbf),
        WGOe=c1go.astype(nbf), WGAe=c1ga.astype(nbf),
        W2=W2v.astype(nbf), CB=CBv, SC=SCv,
        A2OT=A2OTv, O2AT=O2ATv, OUTWT=OUTWTv, LNV=LNVv, FT=FTv,
        EYE=np.eye(P, dtype=nbf),
    )

    in_maps = []
    for c in range(NCORES):
        sl = perms[c]
        tokTv = np.ascontiguousarray(
            tokens[sl].transpose(1, 2, 0)
        ).astype(nbf)                                  # [M, H, BS]
        mv = np.zeros((8, BS), np.float32)
        mv[0:3] = (pv[sl].T > 0)
        mv[3:6] = (mo[sl].T > 0)
        mv[6] = aum[sl] > 0
        mv[7] = ma[sl] > 0
        cmv = np.ascontiguousarray((mo[sl] * winv[sl, None]).T)
        uvv = np.zeros((M, 2, BS), np.float32)
        uvv[:, 0, :] = mask_f[sl].T
        uvv[:, 1, :] = 1.0 - mask_f[sl].T
        in_maps.append(dict(
            tokT=tokTv, mrows=mv.astype(nbf), cmrows=cmv.astype(nbf),
            uv=uvv.astype(nbf), **shared,
        ))
    return in_maps


def _pack_plan(token_mask):
    """Sort batches by mask pattern into per-core columns with identical
    per-pattern quotas across cores (so one compiled NEFF serves all 8),
    plus per-modality active-column spans shared by all cores."""
    mask = np.asarray(token_mask).astype(np.int64)
    code = (mask[:, 0] | (mask[:, 1] << 1) | (mask[:, 2] << 2) | (mask[:, 3] << 3))
    idx_by_p = {p: np.nonzero(code == p)[0] for p in range(16)}
    q = {p: len(idx_by_p[p]) // NCORES for p in range(16)}
    S = (B - NCORES * sum(q.values())) // NCORES  # overflow slots per core
    overflow = np.concatenate(
        [idx_by_p[p][NCORES * q[p]:] for p in PORDER]
    ) if S else np.zeros((0,), np.int64)
    perms = []
    for c in range(NCORES):
        parts = [idx_by_p[p][c * q[p]:(c + 1) * q[p]] for p in PORDER]
        if S:
            parts.append(overflow[c * S:(c + 1) * S])
        perms.append(np.concatenate(parts).astype(np.int64))
        assert perms[-1].shape == (BS,)
    # segment layout (identical across cores): PORDER segments + overflow
    segs = [(p, q[p]) for p in PORDER] + [(-1, S)]  # -1: overflow = all active
    spans = {}
    for m in range(M):
        sp, off = [], 0
        cur = None
        for p, ln in segs:
            act = True if p == -1 else bool((p >> m) & 1)
            if ln:
                if act:
                    if cur is None:
                        cur = [off, off + ln]
                    else:
                        cur[1] = off + ln
                elif cur is not None:
                    sp.append(tuple(cur)); cur = None
            off += ln
        if cur is not None:
            sp.append(tuple(cur))
        spans[m] = tuple(sp)
    return perms, tuple(sorted(spans.items()))


def kernel(**inputs):
    global LAST_RESULTS
    perms, spans_t = _pack_plan(inputs["token_mask"])
    if spans_t not in _cached:
        _cached[spans_t] = _build(dict(spans_t))
    in_maps = _host_prep(inputs, perms)
    res = run_bass_kernel_spmd(
        _cached[spans_t], in_maps, core_ids=list(range(NCORES)), trace=TRACE
    )
    LAST_RESULTS = res
    out = np.zeros((B, H), np.float32)
    for c in range(NCORES):
        out[perms[c]] = res.results[c]["outT"].T
    return np.ascontiguousarray(out)


# revision 16
# speedup vs baseline: 1.0079x; 1.0079x over previous
"""TRN2 Bass kernel for nn_BlendEmoBackbone: gated audio mixer + low-rank
multiplicative fusion, data-parallel over batch on 8 NeuronCores.

Strategy (v2, bf16):
- Pure data parallel: each core handles B/8 = 512 batch rows; the gate MLP
  weights and the LMF factor tensors are replicated.
- All activations kept in transposed [feature, batch] layout on-chip so every
  matmul contracts over the partition dim; tokens are transposed on the host.
- Every matmul operand is bf16 (full PE rate + FWL weight loads + halved
  HBM traffic for the 168MB factor tensor); PSUM accumulation stays f32.
- LayerNorms over the feature dim use PE ones-matmuls for partition sums,
  with the -mu term folded into gate matmuls as an extra contraction row.
- The LMF where(mask, z, 1) and the x_aug ones-column are folded into the
  factor matmul as a K=2 tail tile ([bias_row; ones_row] x [mask; 1-mask]).
- rank_w is folded into the audio factor slices on the host.
- LMF rank accumulation runs on PE via an identity-matmul into a PSUM bank.
- The audio token squares and the s-part of the gate-MLP layer-1 matmul are
  shared across the three "other" gates.
"""

import numpy as np
import ml_dtypes
from contextlib import ExitStack

import concourse.bass as bass
from concourse import bacc
import concourse.tile as tile
from concourse import mybir
from concourse.bass_utils import run_bass_kernel_spmd

B, M, H, R = 4096, 4, 1024, 10
NCORES = 8
BS = B // NCORES          # 512 batch rows per core
MID = 512
P = 128
HT = H // P               # 8 h-tiles
MT = MID // P             # 4 mid-tiles
D3 = 3 * H
OTHERS = (0, 2, 3)
AUDIO = 1
EPS = 1e-5

f32 = mybir.dt.float32
bf16 = mybir.dt.bfloat16
nbf = ml_dtypes.bfloat16
AF = mybir.ActivationFunctionType
OP = mybir.AluOpType

TRACE = False
LAST_RESULTS = None

_cached = {}

# Pattern-group ordering (pattern code bit m = modality m active), chosen so
# each modality's active columns form few contiguous runs; ends at 1111 so
# the overflow segment merges with every modality's last run.
PORDER = [0, 4, 5, 13, 9, 1, 11, 3, 7, 6, 2, 10, 8, 12, 14, 15]


def _build(spans):
    nc = bacc.Bacc("TRN2", target_bir_lowering=False, debug=False)

    # ---- DRAM parameters (per core) ----
    tokT = nc.declare_dram_parameter("tokT", [M, H, BS], bf16, isOutput=False)
    # bf16 mask rows: 0-2 pv_j, 3-5 mo_j, 6 am(aum), 7 ma
    mrows = nc.declare_dram_parameter("mrows", [8, BS], bf16, isOutput=False)
    cmrows = nc.declare_dram_parameter("cmrows", [3, BS], bf16, isOutput=False)
    uv = nc.declare_dram_parameter("uv", [M, 2, BS], bf16, isOutput=False)
    WGO = nc.declare_dram_parameter("WGO", [3 * HT, P, MID], bf16, isOutput=False)
    WGA = nc.declare_dram_parameter("WGA", [3 * HT, P, MID], bf16, isOutput=False)
    WGOe = nc.declare_dram_parameter("WGOe", [1, MID], bf16, isOutput=False)
    WGAe = nc.declare_dram_parameter("WGAe", [1, MID], bf16, isOutput=False)
    W2 = nc.declare_dram_parameter("W2", [P, MT, 2], bf16, isOutput=False)
    CB = nc.declare_dram_parameter("CB", [P, 8], f32, isOutput=False)
    SC = nc.declare_dram_parameter("SC", [1, 8], f32, isOutput=False)
    # [ho, p, kt, c]: wT[kt*P+p, ho*P+c] — one contiguous [P, HT*P] row per ho
    A2OT = nc.declare_dram_parameter("A2OT", [HT, P, HT, P], bf16, isOutput=False)
    O2AT = nc.declare_dram_parameter("O2AT", [HT, P, HT, P], bf16, isOutput=False)
    OUTWT = nc.declare_dram_parameter("OUTWT", [HT, P, HT, P], bf16, isOutput=False)
    # cols: ln_o_w 0:8, ln_o_b 8:16, ln_a_w 16:24, ln_a_b 24:32,
    #       ln1w 32:40, ln1b 40:48, ln2w 48:56, ln2b 56:64, outb 64:72, lmfb 72:80
    LNV = nc.declare_dram_parameter("LNV", [P, 80], f32, isOutput=False)
    # FT[ht, r, m, p, kt, c]: main factor blocks for kt<8; [p,8,:] rows 0/1
    # hold [bias_row; ones_or_rankw_row] for the K=2 tail matmul.
    FT = nc.declare_dram_parameter("FT", [HT, R, P, M, 9, P], bf16, isOutput=False)
    EYE = nc.declare_dram_parameter("EYE", [P, P], bf16, isOutput=False)
    OUT = nc.declare_dram_parameter("outT", [H, BS], f32, isOutput=True)

    with tile.TileContext(nc) as tc, ExitStack() as ctx:
        kp = ctx.enter_context(tc.tile_pool(name="konst", bufs=1))
        tokp = ctx.enter_context(tc.tile_pool(name="tokp", bufs=1))
        big = ctx.enter_context(tc.tile_pool(name="big", bufs=1))
        wk = ctx.enter_context(tc.tile_pool(name="wk", bufs=2))
        bcp = ctx.enter_context(tc.tile_pool(name="bcp", bufs=1))
        sqp = ctx.enter_context(tc.tile_pool(name="sqp", bufs=2))
        wgp = ctx.enter_context(tc.tile_pool(name="wgp", bufs=2))
        ftp = ctx.enter_context(tc.tile_pool(name="ftp", bufs=6))
        rowp = ctx.enter_context(tc.tile_pool(name="rowp", bufs=1))
        ppz = ctx.enter_context(tc.tile_pool(name="ppz", bufs=4, space="PSUM"))
        pps = ctx.enter_context(tc.tile_pool(name="pps", bufs=2, space="PSUM"))
        ppo = ctx.enter_context(tc.tile_pool(name="ppo", bufs=1, space="PSUM"))
        ppa = ctx.enter_context(tc.tile_pool(name="ppa", bufs=1, space="PSUM"))

        # ---- constants / small loads ----
        ones_k = kp.tile([P, 1], bf16)
        nc.vector.memset(ones_k, 1.0)
        ones1 = kp.tile([1, P], bf16)
        nc.vector.memset(ones1, 1.0)
        eye = kp.tile([P, P], bf16)
        nc.sync.dma_start(out=eye, in_=EYE.ap())

        def bc_row_dma(dst, src_ap):
            nc.sync.dma_start(
                out=dst,
                in_=bass.AP(
                    tensor=src_ap.tensor, offset=src_ap.offset, ap=[[0, P], [1, BS]]
                ),
            )

        mt_ = []
        for i in range(8):
            t = kp.tile([P, BS], bf16, tag=f"mk_{i}")
            bc_row_dma(t, mrows.ap()[i : i + 1, :])
            mt_.append(t)
        pv_t, mo_t, am_t, ma_t = mt_[0:3], mt_[3:6], mt_[6], mt_[7]
        cm_t = []
        for i in range(3):
            t = kp.tile([P, BS], bf16, tag=f"cm_{i}")
            bc_row_dma(t, cmrows.ap()[i : i + 1, :])
            cm_t.append(t)
        uvt = []
        for m in range(M):
            t = kp.tile([2, BS], bf16, tag=f"uv_{m}")
            nc.sync.dma_start(out=t, in_=uv.ap()[m])
            uvt.append(t)
        cbt = kp.tile([P, 8], f32)
        nc.sync.dma_start(out=cbt, in_=CB.ap())
        sct = kp.tile([1, 8], f32)
        nc.sync.dma_start(out=sct, in_=SC.ap())
        lnv = kp.tile([P, 80], f32)
        nc.sync.dma_start(out=lnv, in_=LNV.ap())
        w2t = kp.tile([P, MT, 2], bf16)
        nc.sync.dma_start(out=w2t, in_=W2.ap())

        # ---- tokens (transposed) ----
        tok = tokp.tile([P, M, HT, BS], bf16)
        for m in range(M):
            src = tokT.ap()[m].rearrange("(ht p) b -> p ht b", p=P)
            for ht in range(HT):
                nc.sync.dma_start(out=tok[:, m, ht, :], in_=src[:, ht, :])

        def tk(m, kt):
            return tok[:, m, kt, :]

        # ---- helpers ----
        def pe_warm(row, tag):
            """Tiny matmul that depends on a mid-chain row: keeps the PE HAM
            activity window busy during serial LN chains so the clock gate
            stays at 2.4 GHz. ~60 cycles each."""
            dt = ppa.tile([1, 1], f32, tag="acc", name=f"warm_{tag}")
            nc.tensor.matmul(dt, ones_k[0:1, 0:1], row, start=True, stop=True)

        def ln_rows(stat, n, tag):
            """From psum stat banks (sum, sumsq) compute negmu [1,BS] bf16
            and rinv [1,BS] bf16 rows."""
            statA, statB = stat
            negmu = rowp.tile([1, BS], bf16, tag="negmu", name=f"negmu_{tag}")
            nc.scalar.activation(negmu, statA, AF.Copy, bias=0.0, scale=-1.0 / n)
            ex2 = rowp.tile([1, BS], f32, tag="ex2", name=f"ex2_{tag}")
            nc.scalar.activation(ex2, statB, AF.Copy, bias=0.0, scale=1.0 / n)
            msq = rowp.tile([1, BS], f32, tag="msq", name=f"msq_{tag}")
            nc.scalar.activation(msq, negmu, AF.Square)
            pe_warm(negmu[0:1, 0:1], f"a_{tag}")
            nc.vector.tensor_sub(ex2, ex2, msq)                      # var in place
            rinv = rowp.tile([1, BS], bf16, tag="rinv", name=f"rinv_{tag}")
            # rsqrt(v+eps) via abs_reciprocal_sqrt (v+eps > 0 always;
            # plain Rsqrt is guard-blocked in bass).
            nc.scalar.activation(
                rinv, ex2, AF.Abs_reciprocal_sqrt, bias=sct[0:1, 2:3], scale=1.0
            )
            pe_warm(rinv[0:1, 0:1], f"b_{tag}")
            return negmu, rinv

        def bcast(row, tag):
            """Broadcast a [1,BS] bf16 row to a [P,BS] bf16 sbuf tile."""
            po = ppo.tile([P, BS], f32, tag="outer")
            nc.tensor.matmul(po, ones1, row, start=True, stop=True)
            sb = bcp.tile([P, BS], bf16, tag=f"bc_{tag}")
            nc.scalar.activation(sb, po, AF.Copy, bias=0.0, scale=1.0)
            return sb

        def colsum_stats(stat, pairs):
            """Accumulate sum (bank A) and sumsq (bank B) over the given
            (tile, square_tile) pairs of [P,BS] bf16 tiles."""
            statA, statB = stat
            n = len(pairs)
            for i, (t, sq) in enumerate(pairs):
                nc.tensor.matmul(statA, ones_k, t, start=(i == 0), stop=(i == n - 1))
                nc.tensor.matmul(statB, ones_k, sq, start=(i == 0), stop=(i == n - 1))

        # ---- a2o = audio @ a2o_w.T, in T layout [H, BS] ----
        a2or = big.tile([P, HT, BS], bf16, tag="axr")
        for ho in range(HT):
            wt = wgp.tile([P, HT, P], bf16, tag="ww")
            nc.sync.dma_start(out=wt, in_=A2OT.ap()[ho])
            ps = ppz.tile([P, BS], f32, tag="z")
            for kt in range(HT):
                nc.tensor.matmul(ps, wt[:, kt, :], tk(AUDIO, kt), start=(kt == 0), stop=(kt == HT - 1))
            nc.scalar.activation(a2or[:, ho, :], ps, AF.Copy, bias=0.0, scale=1.0)

        # ---- audio squares, shared by gates 0-2 (s) and gate 3 (t) ----
        asq = big.tile([P, HT, BS], bf16, tag="asq")
        for kt in range(HT):
            nc.scalar.activation(asq[:, kt, :], tk(AUDIO, kt), AF.Square)

        # ---- s-part of gate_other layer 1, shared by gates 0-2 ----
        spart = big.tile([P, MT, BS], bf16, tag="spart")
        sps = [ppz.tile([P, BS], f32, tag="z", name=f"sps{mt}") for mt in range(MT)]
        for kt in range(HT):
            wt = wgp.tile([P, MID], bf16, tag="wg")
            nc.sync.dma_start(out=wt, in_=WGO.ap()[HT + kt])
            for mt in range(MT):
                nc.tensor.matmul(
                    sps[mt], wt[:, mt * P : (mt + 1) * P], tk(AUDIO, kt),
                    start=(kt == 0), stop=(kt == HT - 1),
                )
        for mt in range(MT):
            nc.scalar.activation(spart[:, mt, :], sps[mt], AF.Copy, bias=0.0, scale=1.0)

        omt = big.tile([P, HT, BS], bf16, tag="om")  # others_mean accumulator
        mix_src = {"x": a2or}  # a2o for others-gates, o2a for the audio gate

        def gate_and_mix(j, mj):
            """j: 0..2 index into OTHERS, or 3 for the audio gate."""
            is_audio = j == 3
            t_m = AUDIO if is_audio else mj

            def s_tile(kt):
                return omt[:, kt, :] if is_audio else tk(AUDIO, kt)

            # |t-s| tiles + squares + LN stats over the 3H concat features
            _st = pps.tile([33, BS], f32, tag="st", name="statg")
            stat = (_st[0:1, :], _st[32:33, :])
            abs_t = big.tile([P, HT, BS], bf16, tag="abs")
            pairs = []
            for kt in range(HT):
                d = wk.tile([P, BS], bf16, tag="d")
                nc.vector.tensor_sub(d, tk(t_m, kt), s_tile(kt))
                nc.scalar.activation(abs_t[:, kt, :], d, AF.Abs)
                sqd = sqp.tile([P, BS], bf16, tag="sq")
                nc.scalar.activation(sqd, d, AF.Square)
                pairs.append((abs_t[:, kt, :], sqd))
                if is_audio:
                    pairs.append((tk(t_m, kt), asq[:, kt, :]))
                    sqs = sqp.tile([P, BS], bf16, tag="sq")
                    nc.scalar.activation(sqs, s_tile(kt), AF.Square)
                    pairs.append((s_tile(kt), sqs))
                else:
                    sqt = sqp.tile([P, BS], bf16, tag="sq")
                    nc.scalar.activation(sqt, tk(t_m, kt), AF.Square)
                    pairs.append((tk(t_m, kt), sqt))
                    pairs.append((s_tile(kt), asq[:, kt, :]))
            colsum_stats(stat, pairs)
            negmu, rinv = ln_rows(stat, D3, "g")

            # gate layer 1: psum[mt] = W~.T @ [t; s; |t-s|] - mu*c1
            # (s-part pre-computed in `spart` for the non-audio gates)
            WG = WGA if is_audio else WGO
            wge = rowp.tile([1, MID], bf16, tag="wge", name=f"wge{j}")
            nc.sync.dma_start(out=wge, in_=(WGAe if is_audio else WGOe).ap())
            gps = [ppz.tile([P, BS], f32, tag="z", name=f"gps{mt}") for mt in range(MT)]
            kts = range(3 * HT) if is_audio else [k for k in range(3 * HT) if k // HT != 1]
            first = True
            for kt in kts:
                wt = wgp.tile([P, MID], bf16, tag="wg")
                nc.sync.dma_start(out=wt, in_=WG.ap()[kt])
                part, k = kt // HT, kt % HT
                rhs = tk(t_m, k) if part == 0 else (s_tile(k) if part == 1 else abs_t[:, k, :])
                for mt in range(MT):
                    nc.tensor.matmul(
                        gps[mt], wt[:, mt * P : (mt + 1) * P], rhs,
                        start=first, stop=False,
                    )
                first = False
            for mt in range(MT):
                nc.tensor.matmul(
                    gps[mt], wge[0:1, mt * P : (mt + 1) * P], negmu,
                    start=False, stop=True,
                )
            rb = bcast(rinv, "rb")
            cb_off = 4 if is_audio else 0
            col = 1 if is_audio else 0
            _gp = pps.tile([33, BS], f32, tag="st", name="gp")
            gp = _gp[0:1, :]
            for mt in range(MT):
                hm = wk.tile([P, BS], bf16, tag="hm")
                if is_audio:
                    nc.vector.tensor_mul(hm, gps[mt], rb)
                else:
                    nc.vector.tensor_add(hm, gps[mt], spart[:, mt, :])
                    nc.vector.tensor_mul(hm, hm, rb)
                hg1 = wk.tile([P, BS], bf16, tag="hg", name=f"hg{mt}")
                nc.scalar.activation(
                    hg1, hm, AF.Gelu,
                    bias=cbt[:, cb_off + mt : cb_off + mt + 1], scale=1.0,
                )
                nc.tensor.matmul(
                    gp, w2t[:, mt, col : col + 1], hg1,
                    start=(mt == 0), stop=(mt == MT - 1),
                )
            g_row = rowp.tile([1, BS], bf16, tag="g_row")
            nc.scalar.activation(
                g_row, gp, AF.Sigmoid,
                bias=sct[0:1, col : col + 1], scale=1.0,
            )
            pe_warm(g_row[0:1, 0:1], f"g{j}")
            gb = bcast(g_row, "gb")

            # pre = t + g * (a2o | o2a); LN over H; blend into tok in place
            src = mix_src["x"]
            pre = big.tile([P, HT, BS], bf16, tag="abs", name="pre")
            _st2 = pps.tile([33, BS], f32, tag="st", name="stat2")
            stat2 = (_st2[0:1, :], _st2[32:33, :])
            pairs2 = []
            for kt in range(HT):
                tmp = wk.tile([P, BS], bf16, tag="hm")
                nc.vector.tensor_mul(tmp, gb, src[:, kt, :])
                nc.vector.tensor_add(pre[:, kt, :], tmp, tk(t_m, kt))
                sq = sqp.tile([P, BS], bf16, tag="sq")
                nc.scalar.activation(sq, pre[:, kt, :], AF.Square)
                pairs2.append((pre[:, kt, :], sq))
            colsum_stats(stat2, pairs2)
            negmu2, rinv2 = ln_rows(stat2, H, "u")
            mb = bcast(negmu2, "mb")
            rb2 = bcast(rinv2, "rb2")
            wcol = 16 if is_audio else 0
            bcol = 24 if is_audio else 8
            for kt in range(HT):
                nc.vector.tensor_add(pre[:, kt, :], pre[:, kt, :], mb)
                nc.vector.tensor_mul(pre[:, kt, :], pre[:, kt, :], rb2)
                nc.vector.tensor_scalar(
                    pre[:, kt, :], pre[:, kt, :],
                    lnv[:, wcol + kt : wcol + kt + 1], lnv[:, bcol + kt : bcol + kt + 1],
                    op0=OP.mult, op1=OP.add,
                )
                # blend = big_mask*t + small_mask*(upd - t), in place into tok
                bm = ma_t if is_audio else mo_t[j]
                sm = am_t if is_audio else pv_t[j]
                d2 = wk.tile([P, BS], bf16, tag="d", name="d2")
                if kt % 2 == 0:
                    pe_warm(pre[0:1, kt, 0:1], f"bl{j}_{kt}")
                nc.vector.tensor_sub(d2, pre[:, kt, :], tk(t_m, kt))
                nc.vector.tensor_mul(d2, d2, sm)
                nc.vector.tensor_mul(tk(t_m, kt), tk(t_m, kt), bm)
                nc.vector.tensor_add(tk(t_m, kt), tk(t_m, kt), d2)
                if not is_audio:
                    if j == 0:
                        nc.vector.tensor_mul(omt[:, kt, :], cm_t[j], tk(mj, kt))
                    else:
                        tmp2 = wk.tile([P, BS], bf16, tag="hm")
                        nc.vector.tensor_mul(tmp2, cm_t[j], tk(mj, kt))
                        nc.vector.tensor_add(omt[:, kt, :], omt[:, kt, :], tmp2)

        for j, mj in enumerate(OTHERS):
            gate_and_mix(j, mj)

        # ---- o2a = others_mean @ o2a_w.T ----
        o2ar = big.tile([P, HT, BS], bf16, tag="axr")
        for ho in range(HT):
            wt = wgp.tile([P, HT, P], bf16, tag="ww")
            nc.sync.dma_start(out=wt, in_=O2AT.ap()[ho])
            ps = ppz.tile([P, BS], f32, tag="z")
            for kt in range(HT):
                nc.tensor.matmul(ps, wt[:, kt, :], omt[:, kt, :], start=(kt == 0), stop=(kt == HT - 1))
            nc.scalar.activation(o2ar[:, ho, :], ps, AF.Copy, bias=0.0, scale=1.0)
        mix_src["x"] = o2ar

        gate_and_mix(3, AUDIO)

        # ---- LMF: z per (ht,r,m); prod over m on DVE; rank-sum on PE ----
        accs = big.tile([P, HT, BS], bf16, tag="accs")
        for ht in range(HT):
            acc_ps = ppa.tile([P, BS], f32, tag="acc", name=f"acc{ht}")
            for r in range(R):
                zps = []
                s01 = None
                z0c = None
                ftm = ftp.tile([P, M, 9, P], bf16, tag="ft")
                nc.sync.dma_start(out=ftm, in_=FT.ap()[ht, r])
                for m in range(M):
                    zp = ppz.tile([P, BS], f32, tag="z")
                    # tail first (full width): bias*mask + 1*(1-mask); main
                    # matmuls then accumulate only over active column spans.
                    sp = spans[m]
                    nc.tensor.matmul(
                        zp, ftm[0:2, m, 8, :], uvt[m], start=True, stop=(len(sp) == 0)
                    )
                    for kt in range(HT):
                        for si, (a, b) in enumerate(sp):
                            inst = nc.tensor.matmul(
                                zp[:, a:b], ftm[:, m, kt, :], tok[:, m, kt, a:b],
                                start=False,
                                stop=(kt == HT - 1 and si == len(sp) - 1),
                            )
                            if si > 0:
                                # same stationary as the previous span matmul —
                                # skip the redundant LDWEIGHTS
                                inst.ins.ldweights = False
                    zps.append(zp)
                    # PSUM has one DVE read port: stage one factor of each
                    # pair through SBUF via the (otherwise idle) ScalarE.
                    if m == 0:
                        z0c = wk.tile([P, BS], bf16, tag="s01", name="z0c")
                        nc.scalar.activation(z0c, zps[0], AF.Copy, bias=0.0, scale=1.0)
                    if m == 1:
                        s01 = wk.tile([P, BS], bf16, tag="s01", name="s01")
                        nc.vector.tensor_mul(s01, z0c, zps[1])
                    if m == 2:
                        z0c = wk.tile([P, BS], bf16, tag="s23", name="z2c")
                        nc.scalar.activation(z0c, zps[2], AF.Copy, bias=0.0, scale=1.0)
                s23 = wk.tile([P, BS], bf16, tag="s23", name="s23")
                nc.vector.tensor_mul(s23, z0c, zps[3])
                prod = wk.tile([P, BS], bf16, tag="prod")
                nc.vector.tensor_mul(prod, s01, s23)
                nc.tensor.matmul(acc_ps, eye, prod, start=(r == 0), stop=(r == R - 1))
            # accs = acc + lmf_bias (bias col folded into the copy)
            nc.scalar.activation(
                accs[:, ht, :], acc_ps, AF.Identity,
                bias=lnv[:, 72 + ht : 72 + ht + 1], scale=1.0,
            )

        # ---- output MLP ----
        _st3 = pps.tile([33, BS], f32, tag="st", name="stat3")
        stat3 = (_st3[0:1, :], _st3[32:33, :])
        pairs3 = []
        for kt in range(HT):
            sq = sqp.tile([P, BS], bf16, tag="sq")
            nc.scalar.activation(sq, accs[:, kt, :], AF.Square)
            pairs3.append((accs[:, kt, :], sq))
        colsum_stats(stat3, pairs3)
        negmu3, rinv3 = ln_rows(stat3, H, "l1")
        mb3 = bcast(negmu3, "mb")
        rb3 = bcast(rinv3, "rb2")
        for kt in range(HT):
            nc.vector.tensor_add(accs[:, kt, :], accs[:, kt, :], mb3)
            nc.vector.tensor_mul(accs[:, kt, :], accs[:, kt, :], rb3)
            nc.vector.tensor_scalar(
                accs[:, kt, :], accs[:, kt, :],
                lnv[:, 32 + kt : 32 + kt + 1], lnv[:, 40 + kt : 40 + kt + 1],
                op0=OP.mult, op1=OP.add,
            )

        # h2 = gelu(h1 @ out_w.T + out_b); LN2; write out
        h2 = big.tile([P, HT, BS], bf16, tag="abs")
        _st4 = pps.tile([33, BS], f32, tag="st", name="stat4")
        stat4 = (_st4[0:1, :], _st4[32:33, :])
        pairs4 = []
        for ho in range(HT):
            wt = wgp.tile([P, HT, P], bf16, tag="ww")
            nc.sync.dma_start(out=wt, in_=OUTWT.ap()[ho])
            ps = ppz.tile([P, BS], f32, tag="z")
            for kt in range(HT):
                nc.tensor.matmul(ps, wt[:, kt, :], accs[:, kt, :], start=(kt == 0), stop=(kt == HT - 1))
            nc.scalar.activation(
                h2[:, ho, :], ps, AF.Gelu, bias=lnv[:, 64 + ho : 64 + ho + 1], scale=1.0
            )
            sq = sqp.tile([P, BS], bf16, tag="sq")
            nc.scalar.activation(sq, h2[:, ho, :], AF.Square)
            pairs4.append((h2[:, ho, :], sq))
        colsum_stats(stat4, pairs4)
        negmu4, rinv4 = ln_rows(stat4, H, "l2")
        mb4 = bcast(negmu4, "mb")
        rb4 = bcast(rinv4, "rb2")
        for kt in range(HT):
            fin = wk.tile([P, BS], f32, tag="fin")
            nc.vector.tensor_add(fin, h2[:, kt, :], mb4)
            nc.vector.tensor_mul(fin, fin, rb4)
            nc.vector.tensor_scalar(
                fin, fin, lnv[:, 48 + kt : 48 + kt + 1], lnv[:, 56 + kt : 56 + kt + 1],
                op0=OP.mult, op1=OP.add,
            )
            nc.sync.dma_start(out=OUT.ap()[kt * P : (kt + 1) * P, :], in_=fin)

    nc.compile()
    return nc


def _host_prep(inputs, perms):
    tokens = np.asarray(inputs["tokens"], np.float32)
    token_mask = np.asarray(inputs["token_mask"])
    mask_f = token_mask.astype(np.float32)

    mo = mask_f[:, list(OTHERS)]                      # [B,3]
    ma = mask_f[:, AUDIO]                             # [B]
    pv = mo * ma[:, None]                             # [B,3]
    winv = (1.0 / np.clip(mo.sum(1), 1.0, None)).astype(np.float32)
    aum = ma * (mo.max(1) > 0)                        # [B]

    go_w1 = np.asarray(inputs["go_w1"], np.float32)
    ga_w1 = np.asarray(inputs["ga_w1"], np.float32)

    def gate_prep(w1, b1, lnw, lnb):
        W1w = w1 * lnw[None, :]                       # [MID, 3H]
        c1 = np.ascontiguousarray(W1w.sum(1).reshape(1, MID))
        cb = w1 @ lnb + b1                            # [MID]
        Wblocks = np.ascontiguousarray(W1w.T).reshape(3 * HT, P, MID)
        return Wblocks, c1, cb

    WGOv, c1go, cbgo = gate_prep(
        go_w1, np.asarray(inputs["go_b1"], np.float32),
        np.asarray(inputs["ln_go_w"], np.float32), np.asarray(inputs["ln_go_b"], np.float32),
    )
    WGAv, c1ga, cbga = gate_prep(
        ga_w1, np.asarray(inputs["ga_b1"], np.float32),
        np.asarray(inputs["ln_ga_w"], np.float32), np.asarray(inputs["ln_ga_b"], np.float32),
    )
    CBv = np.ascontiguousarray(
        np.concatenate([cbgo.reshape(MT, P).T, cbga.reshape(MT, P).T], axis=1)
    ).astype(np.float32)                              # [P, 8]
    W2v = np.stack(
        [np.asarray(inputs["go_w2"], np.float32).reshape(MID),
         np.asarray(inputs["ga_w2"], np.float32).reshape(MID)], axis=1
    )                                                 # [MID, 2]
    W2v = np.ascontiguousarray(W2v.reshape(MT, P, 2).transpose(1, 0, 2))
    SCv = np.zeros((1, 8), np.float32)
    SCv[0, 0] = np.asarray(inputs["go_b2"], np.float32).reshape(-1)[0]
    SCv[0, 1] = np.asarray(inputs["ga_b2"], np.float32).reshape(-1)[0]
    SCv[0, 2] = EPS

    def tile_blocks(w):
        wt = np.ascontiguousarray(np.asarray(w, np.float32).T)    # [H_in, H_out]
        # [ho, p, kt, c] = wT[kt*P+p, ho*P+c]
        return np.ascontiguousarray(
            wt.reshape(HT, P, HT, P).transpose(2, 1, 0, 3)
        ).astype(nbf)

    A2OTv = tile_blocks(inputs["a2o_w"])
    O2ATv = tile_blocks(inputs["o2a_w"])
    OUTWTv = tile_blocks(inputs["out_w"])

    def cols(name):
        return np.asarray(inputs[name], np.float32).reshape(HT, P).T

    LNVv = np.zeros((P, 80), np.float32)
    for i, name in enumerate(
        ["ln_o_w", "ln_o_b", "ln_a_w", "ln_a_b", "out_ln1_w", "out_ln1_b",
         "out_ln2_w", "out_ln2_b", "out_b", "lmf_bias"]
    ):
        LNVv[:, 8 * i : 8 * (i + 1)] = cols(name)

    factors = np.asarray(inputs["factors"], np.float32)
    rank_w = np.asarray(inputs["rank_w"], np.float32)
    Ff = factors.copy()
    Ff[AUDIO] = Ff[AUDIO] * rank_w[:, None, None]
    FTv = np.zeros((HT, R, P, M, 9, P), np.float32)
    # main [m, r, kt, pk, ht, ph] -> [ht, r, pk, m, kt, ph]
    main = Ff[:, :, 1:, :].reshape(M, R, HT, P, HT, P)
    FTv[:, :, :, :, :8, :] = main.transpose(4, 1, 3, 0, 2, 5)
    bias = Ff[:, :, 0, :].reshape(M, R, HT, P)           # [m, r, ht, ph]
    FTv[:, :, 0, :, 8, :] = bias.transpose(2, 1, 0, 3)
    ones_row = np.ones((HT, R, M, P), np.float32)
    ones_row[:, :, AUDIO, :] = rank_w[None, :, None]
    FTv[:, :, 1, :, 8, :] = ones_row
    FTv = FTv.astype(nbf)

    shared = dict(
        WGO=WGOv.astype(nbf), WGA=WGAv.astype(nbf),
        WGOe=c1go.astype(nbf), WGAe=c1ga.astype(nbf),
        W2=W2v.astype(nbf), CB=CBv, SC=SCv,
        A2OT=A2OTv, O2AT=O2ATv, OUTWT=OUTWTv, LNV=LNVv, FT=FTv,
        EYE=np.eye(P, dtype=nbf),
    )

    in_maps = []
    for c in range(NCORES):
        sl = perms[c]
        tokTv = np.ascontiguousarray(
            tokens[sl].transpose(1, 2, 0)
        ).astype(nbf)                                  # [M, H, BS]
        mv = np.zeros((8, BS), np.float32)
        mv[0:3] = (pv[sl].T > 0)
        mv[3:6] = (mo[sl].T > 0)
        mv[6] = aum[sl] > 0
        mv[7] = ma[sl] > 0
        cmv = np.ascontiguousarray((mo[sl] * winv[sl, None]).T)
        uvv = np.zeros((M, 2, BS), np.float32)
        uvv[:, 0, :] = mask_f[sl].T
        uvv[:, 1, :] = 1.0 - mask_f[sl].T
        in_maps.append(dict(
            tokT=tokTv, mrows=mv.astype(nbf), cmrows=cmv.astype(nbf),
            uv=uvv.astype(nbf), **shared,
        ))
    return in_maps


def _pack_plan(token_mask):
    """Sort batches by mask pattern into per-core columns with identical
    per-pattern quotas across cores (so one compiled NEFF serves all 8),
    plus per-modality active-column spans shared by all cores."""
    mask = np.asarray(token_mask).astype(np.int64)
    code = (mask[:, 0] | (mask[:, 1] << 1) | (mask[:, 2] << 2) | (mask[:, 3] << 3))
    idx_by_p = {p: np.nonzero(code == p)[0] for p in range(16)}
    q = {p: len(idx_by_p[p]) // NCORES for p in range(16)}
    S = (B - NCORES * sum(q.values())) // NCORES  # overflow slots per core
    overflow = np.concatenate(
        [idx_by_p[p][NCORES * q[p]:] for p in PORDER]
    ) if S else np.zeros((0,), np.int64)
    perms = []
    for c in range(NCORES):
        parts = [idx_by_p[p][c * q[p]:(c + 1) * q[p]] for p in PORDER]
        if S:
            parts.append(overflow[c * S:(c + 1) * S])
        perms.append(np.concatenate(parts).astype(np.int64))
        assert perms[-1].shape == (BS,)
    # segment layout (identical across cores): PORDER segments + overflow
    segs = [(p, q[p]) for p in PORDER] + [(-1, S)]  # -1: overflow = all active
    spans = {}
    for m in range(M):
        sp, off = [], 0
        cur = None
        for p, ln in segs:
            act = True if p == -1 else bool((p >> m) & 1)
            if ln:
                if act:
                    if cur is None:
                        cur = [off, off + ln]
                    else:
                        cur[1] = off + ln
                elif cur is not None:
                    sp.append(tuple(cur)); cur = None
            off += ln
        if cur is not None:
            sp.append(tuple(cur))
        spans[m] = tuple(sp)
    return perms, tuple(sorted(spans.items()))


def kernel(**inputs):
    global LAST_RESULTS
    perms, spans_t = _pack_plan(inputs["token_mask"])
    if spans_t not in _cached:
        _cached[spans_t] = _build(dict(spans_t))
    in_maps = _host_prep(inputs, perms)
    res = run_bass_kernel_spmd(
        _cached[spans_t], in_maps, core_ids=list(range(NCORES)), trace=TRACE
    )
    LAST_RESULTS = res
    out = np.zeros((B, H), np.float32)
    for c in range(NCORES):
        out[perms[c]] = res.results[c]["outT"].T
    return np.ascontiguousarray(out)


# revision 17
# speedup vs baseline: 1.0428x; 1.0346x over previous
"""TRN2 Bass kernel for nn_BlendEmoBackbone: gated audio mixer + low-rank
multiplicative fusion, data-parallel over batch on 8 NeuronCores.

Strategy (v2, bf16):
- Pure data parallel: each core handles B/8 = 512 batch rows; the gate MLP
  weights and the LMF factor tensors are replicated.
- All activations kept in transposed [feature, batch] layout on-chip so every
  matmul contracts over the partition dim; tokens are transposed on the host.
- Every matmul operand is bf16 (full PE rate + FWL weight loads + halved
  HBM traffic for the 168MB factor tensor); PSUM accumulation stays f32.
- LayerNorms over the feature dim use PE ones-matmuls for partition sums,
  with the -mu term folded into gate matmuls as an extra contraction row.
- The LMF where(mask, z, 1) and the x_aug ones-column are folded into the
  factor matmul as a K=2 tail tile ([bias_row; ones_row] x [mask; 1-mask]).
- rank_w is folded into the audio factor slices on the host.
- LMF rank accumulation runs on PE via an identity-matmul into a PSUM bank.
- The audio token squares and the s-part of the gate-MLP layer-1 matmul are
  shared across the three "other" gates.
"""

import numpy as np
import ml_dtypes
from contextlib import ExitStack

import concourse.bass as bass
from concourse import bacc
import concourse.tile as tile
from concourse import mybir
from concourse.bass_utils import run_bass_kernel_spmd

B, M, H, R = 4096, 4, 1024, 10
NCORES = 8
BS = B // NCORES          # 512 batch rows per core
MID = 512
P = 128
HT = H // P               # 8 h-tiles
MT = MID // P             # 4 mid-tiles
D3 = 3 * H
OTHERS = (0, 2, 3)
AUDIO = 1
EPS = 1e-5

f32 = mybir.dt.float32
bf16 = mybir.dt.bfloat16
nbf = ml_dtypes.bfloat16
AF = mybir.ActivationFunctionType
OP = mybir.AluOpType

TRACE = False
LAST_RESULTS = None

_cached = {}

# Pattern-group ordering (pattern code bit m = modality m active), chosen so
# each modality's active columns form few contiguous runs; ends at 1111 so
# the overflow segment merges with every modality's last run.
PORDER = [0, 4, 5, 13, 9, 1, 11, 3, 7, 6, 2, 10, 8, 12, 14, 15]


def _build(spans):
    nc = bacc.Bacc("TRN2", target_bir_lowering=False, debug=False)

    # ---- DRAM parameters (per core) ----
    tokT = nc.declare_dram_parameter("tokT", [M, H, BS], bf16, isOutput=False)
    # bf16 mask rows: 0-2 pv_j, 3-5 mo_j, 6 am(aum), 7 ma
    mrows = nc.declare_dram_parameter("mrows", [8, BS], bf16, isOutput=False)
    cmrows = nc.declare_dram_parameter("cmrows", [3, BS], bf16, isOutput=False)
    uv = nc.declare_dram_parameter("uv", [M, 2, BS], bf16, isOutput=False)
    WGO = nc.declare_dram_parameter("WGO", [3 * HT, P, MID], bf16, isOutput=False)
    WGA = nc.declare_dram_parameter("WGA", [3 * HT, P, MID], bf16, isOutput=False)
    WGOe = nc.declare_dram_parameter("WGOe", [1, MID], bf16, isOutput=False)
    WGAe = nc.declare_dram_parameter("WGAe", [1, MID], bf16, isOutput=False)
    W2 = nc.declare_dram_parameter("W2", [P, MT, 2], bf16, isOutput=False)
    CB = nc.declare_dram_parameter("CB", [P, 8], f32, isOutput=False)
    SC = nc.declare_dram_parameter("SC", [1, 8], f32, isOutput=False)
    # [ho, p, kt, c]: wT[kt*P+p, ho*P+c] — one contiguous [P, HT*P] row per ho
    A2OT = nc.declare_dram_parameter("A2OT", [HT, P, HT, P], bf16, isOutput=False)
    O2AT = nc.declare_dram_parameter("O2AT", [HT, P, HT, P], bf16, isOutput=False)
    OUTWT = nc.declare_dram_parameter("OUTWT", [HT, P, HT, P], bf16, isOutput=False)
    # cols: ln_o_w 0:8, ln_o_b 8:16, ln_a_w 16:24, ln_a_b 24:32,
    #       ln1w 32:40, ln1b 40:48, ln2w 48:56, ln2b 56:64, outb 64:72, lmfb 72:80
    LNV = nc.declare_dram_parameter("LNV", [P, 80], f32, isOutput=False)
    # FT[ht, r, m, p, kt, c]: main factor blocks for kt<8; [p,8,:] rows 0/1
    # hold [bias_row; ones_or_rankw_row] for the K=2 tail matmul.
    FT = nc.declare_dram_parameter("FT", [HT, R, P, M, 9, P], bf16, isOutput=False)
    EYE = nc.declare_dram_parameter("EYE", [P, P], bf16, isOutput=False)
    OUT = nc.declare_dram_parameter("outT", [H, BS], f32, isOutput=True)

    with tile.TileContext(nc) as tc, ExitStack() as ctx:
        kp = ctx.enter_context(tc.tile_pool(name="konst", bufs=1))
        tokp = ctx.enter_context(tc.tile_pool(name="tokp", bufs=1))
        big = ctx.enter_context(tc.tile_pool(name="big", bufs=1))
        wk = ctx.enter_context(tc.tile_pool(name="wk", bufs=3))
        bcp = ctx.enter_context(tc.tile_pool(name="bcp", bufs=1))
        sqp = ctx.enter_context(tc.tile_pool(name="sqp", bufs=4))
        wgp = ctx.enter_context(tc.tile_pool(name="wgp", bufs=2))
        ftp = ctx.enter_context(tc.tile_pool(name="ftp", bufs=6))
        rowp = ctx.enter_context(tc.tile_pool(name="rowp", bufs=1))
        ppz = ctx.enter_context(tc.tile_pool(name="ppz", bufs=4, space="PSUM"))
        pps = ctx.enter_context(tc.tile_pool(name="pps", bufs=2, space="PSUM"))
        ppo = ctx.enter_context(tc.tile_pool(name="ppo", bufs=1, space="PSUM"))
        ppa = ctx.enter_context(tc.tile_pool(name="ppa", bufs=1, space="PSUM"))

        # ---- constants / small loads ----
        ones_k = kp.tile([P, 1], bf16)
        nc.vector.memset(ones_k, 1.0)
        ones1 = kp.tile([1, P], bf16)
        nc.vector.memset(ones1, 1.0)
        eye = kp.tile([P, P], bf16)
        nc.sync.dma_start(out=eye, in_=EYE.ap())

        def bc_row_dma(dst, src_ap):
            nc.sync.dma_start(
                out=dst,
                in_=bass.AP(
                    tensor=src_ap.tensor, offset=src_ap.offset, ap=[[0, P], [1, BS]]
                ),
            )

        mt_ = []
        for i in range(8):
            t = kp.tile([P, BS], bf16, tag=f"mk_{i}")
            bc_row_dma(t, mrows.ap()[i : i + 1, :])
            mt_.append(t)
        pv_t, mo_t, am_t, ma_t = mt_[0:3], mt_[3:6], mt_[6], mt_[7]
        cm_t = []
        for i in range(3):
            t = kp.tile([P, BS], bf16, tag=f"cm_{i}")
            bc_row_dma(t, cmrows.ap()[i : i + 1, :])
            cm_t.append(t)
        uvt = []
        for m in range(M):
            t = kp.tile([2, BS], bf16, tag=f"uv_{m}")
            nc.sync.dma_start(out=t, in_=uv.ap()[m])
            uvt.append(t)
        cbt = kp.tile([P, 8], f32)
        nc.sync.dma_start(out=cbt, in_=CB.ap())
        sct = kp.tile([1, 8], f32)
        nc.sync.dma_start(out=sct, in_=SC.ap())
        lnv = kp.tile([P, 80], f32)
        nc.sync.dma_start(out=lnv, in_=LNV.ap())
        w2t = kp.tile([P, MT, 2], bf16)
        nc.sync.dma_start(out=w2t, in_=W2.ap())

        # ---- tokens (transposed) ----
        tok = tokp.tile([P, M, HT, BS], bf16)
        for m in range(M):
            src = tokT.ap()[m].rearrange("(ht p) b -> p ht b", p=P)
            for ht in range(HT):
                nc.sync.dma_start(out=tok[:, m, ht, :], in_=src[:, ht, :])

        def tk(m, kt):
            return tok[:, m, kt, :]

        # ---- helpers ----
        def pe_warm(row, tag):
            """Tiny matmul that depends on a mid-chain row: keeps the PE HAM
            activity window busy during serial LN chains so the clock gate
            stays at 2.4 GHz. ~60 cycles each."""
            dt = ppa.tile([1, 1], f32, tag="acc", name=f"warm_{tag}")
            nc.tensor.matmul(dt, ones_k[0:1, 0:1], row, start=True, stop=True)

        def ln_rows(stat, n, tag):
            """From psum stat banks (sum, sumsq) compute negmu [1,BS] bf16
            and rinv [1,BS] bf16 rows."""
            statA, statB = stat
            negmu = rowp.tile([1, BS], bf16, tag="negmu", name=f"negmu_{tag}")
            nc.scalar.activation(negmu, statA, AF.Copy, bias=0.0, scale=-1.0 / n)
            ex2 = rowp.tile([1, BS], f32, tag="ex2", name=f"ex2_{tag}")
            nc.scalar.activation(ex2, statB, AF.Copy, bias=0.0, scale=1.0 / n)
            msq = rowp.tile([1, BS], f32, tag="msq", name=f"msq_{tag}")
            nc.scalar.activation(msq, negmu, AF.Square)
            pe_warm(negmu[0:1, 0:1], f"a_{tag}")
            nc.vector.tensor_sub(ex2, ex2, msq)                      # var in place
            rinv = rowp.tile([1, BS], bf16, tag="rinv", name=f"rinv_{tag}")
            # rsqrt(v+eps) via abs_reciprocal_sqrt (v+eps > 0 always;
            # plain Rsqrt is guard-blocked in bass).
            nc.scalar.activation(
                rinv, ex2, AF.Abs_reciprocal_sqrt, bias=sct[0:1, 2:3], scale=1.0
            )
            pe_warm(rinv[0:1, 0:1], f"b_{tag}")
            return negmu, rinv

        def bcast(row, tag):
            """Broadcast a [1,BS] bf16 row to a [P,BS] bf16 sbuf tile."""
            po = ppo.tile([P, BS], f32, tag="outer")
            nc.tensor.matmul(po, ones1, row, start=True, stop=True)
            sb = bcp.tile([P, BS], bf16, tag=f"bc_{tag}")
            nc.scalar.activation(sb, po, AF.Copy, bias=0.0, scale=1.0)
            return sb

        def colsum_stats(stat, pairs):
            """Accumulate sum (bank A) and sumsq (bank B) over the given
            (tile, square_tile) pairs of [P,BS] bf16 tiles."""
            statA, statB = stat
            n = len(pairs)
            for i, (t, sq) in enumerate(pairs):
                nc.tensor.matmul(statA, ones_k, t, start=(i == 0), stop=(i == n - 1))
                nc.tensor.matmul(statB, ones_k, sq, start=(i == 0), stop=(i == n - 1))

        # ---- a2o = audio @ a2o_w.T, in T layout [H, BS] ----
        a2or = big.tile([P, HT, BS], bf16, tag="axr")
        for ho in range(HT):
            wt = wgp.tile([P, HT, P], bf16, tag="ww")
            nc.sync.dma_start(out=wt, in_=A2OT.ap()[ho])
            ps = ppz.tile([P, BS], f32, tag="z")
            for kt in range(HT):
                nc.tensor.matmul(ps, wt[:, kt, :], tk(AUDIO, kt), start=(kt == 0), stop=(kt == HT - 1))
            nc.scalar.activation(a2or[:, ho, :], ps, AF.Copy, bias=0.0, scale=1.0)

        # ---- audio squares, shared by gates 0-2 (s) and gate 3 (t) ----
        asq = big.tile([P, HT, BS], bf16, tag="asq")
        for kt in range(HT):
            nc.scalar.activation(asq[:, kt, :], tk(AUDIO, kt), AF.Square)

        # ---- s-part of gate_other layer 1, shared by gates 0-2 ----
        spart = big.tile([P, MT, BS], bf16, tag="spart")
        sps = [ppz.tile([P, BS], f32, tag="z", name=f"sps{mt}") for mt in range(MT)]
        for kt in range(HT):
            wt = wgp.tile([P, MID], bf16, tag="wg")
            nc.sync.dma_start(out=wt, in_=WGO.ap()[HT + kt])
            for mt in range(MT):
                nc.tensor.matmul(
                    sps[mt], wt[:, mt * P : (mt + 1) * P], tk(AUDIO, kt),
                    start=(kt == 0), stop=(kt == HT - 1),
                )
        for mt in range(MT):
            nc.scalar.activation(spart[:, mt, :], sps[mt], AF.Copy, bias=0.0, scale=1.0)

        omt = big.tile([P, HT, BS], bf16, tag="om")  # others_mean accumulator
        mix_src = {"x": a2or}  # a2o for others-gates, o2a for the audio gate

        def gate_and_mix(j, mj):
            """j: 0..2 index into OTHERS, or 3 for the audio gate."""
            is_audio = j == 3
            t_m = AUDIO if is_audio else mj

            def s_tile(kt):
                return omt[:, kt, :] if is_audio else tk(AUDIO, kt)

            # |t-s| tiles + squares + LN stats over the 3H concat features
            _st = pps.tile([33, BS], f32, tag="st", name="statg")
            stat = (_st[0:1, :], _st[32:33, :])
            abs_t = big.tile([P, HT, BS], bf16, tag="abs")
            pairs = []
            for kt in range(HT):
                d = wk.tile([P, BS], bf16, tag="d")
                nc.vector.tensor_sub(d, tk(t_m, kt), s_tile(kt))
                nc.scalar.activation(abs_t[:, kt, :], d, AF.Abs)
                sqd = sqp.tile([P, BS], bf16, tag="sq")
                nc.scalar.activation(sqd, d, AF.Square)
                pairs.append((abs_t[:, kt, :], sqd))
                if is_audio:
                    pairs.append((tk(t_m, kt), asq[:, kt, :]))
                    sqs = sqp.tile([P, BS], bf16, tag="sq")
                    nc.scalar.activation(sqs, s_tile(kt), AF.Square)
                    pairs.append((s_tile(kt), sqs))
                else:
                    sqt = sqp.tile([P, BS], bf16, tag="sq")
                    nc.scalar.activation(sqt, tk(t_m, kt), AF.Square)
                    pairs.append((tk(t_m, kt), sqt))
                    pairs.append((s_tile(kt), asq[:, kt, :]))
            colsum_stats(stat, pairs)
            negmu, rinv = ln_rows(stat, D3, "g")

            # gate layer 1: psum[mt] = W~.T @ [t; s; |t-s|] - mu*c1
            # (s-part pre-computed in `spart` for the non-audio gates)
            WG = WGA if is_audio else WGO
            wge = rowp.tile([1, MID], bf16, tag="wge", name=f"wge{j}")
            nc.sync.dma_start(out=wge, in_=(WGAe if is_audio else WGOe).ap())
            gps = [ppz.tile([P, BS], f32, tag="z", name=f"gps{mt}") for mt in range(MT)]
            kts = range(3 * HT) if is_audio else [k for k in range(3 * HT) if k // HT != 1]
            first = True
            for kt in kts:
                wt = wgp.tile([P, MID], bf16, tag="wg")
                nc.sync.dma_start(out=wt, in_=WG.ap()[kt])
                part, k = kt // HT, kt % HT
                rhs = tk(t_m, k) if part == 0 else (s_tile(k) if part == 1 else abs_t[:, k, :])
                for mt in range(MT):
                    nc.tensor.matmul(
                        gps[mt], wt[:, mt * P : (mt + 1) * P], rhs,
                        start=first, stop=False,
                    )
                first = False
            for mt in range(MT):
                nc.tensor.matmul(
                    gps[mt], wge[0:1, mt * P : (mt + 1) * P], negmu,
                    start=False, stop=True,
                )
            rb = bcast(rinv, "rb")
            cb_off = 4 if is_audio else 0
            col = 1 if is_audio else 0
            _gp = pps.tile([33, BS], f32, tag="st", name="gp")
            gp = _gp[0:1, :]
            for mt in range(MT):
                hm = wk.tile([P, BS], bf16, tag="hm")
                if is_audio:
                    nc.vector.tensor_mul(hm, gps[mt], rb)
                else:
                    nc.vector.tensor_add(hm, gps[mt], spart[:, mt, :])
                    nc.vector.tensor_mul(hm, hm, rb)
                hg1 = wk.tile([P, BS], bf16, tag="hg", name=f"hg{mt}")
                nc.scalar.activation(
                    hg1, hm, AF.Gelu,
                    bias=cbt[:, cb_off + mt : cb_off + mt + 1], scale=1.0,
                )
                nc.tensor.matmul(
                    gp, w2t[:, mt, col : col + 1], hg1,
                    start=(mt == 0), stop=(mt == MT - 1),
                )
            g_row = rowp.tile([1, BS], bf16, tag="g_row")
            nc.scalar.activation(
                g_row, gp, AF.Sigmoid,
                bias=sct[0:1, col : col + 1], scale=1.0,
            )
            pe_warm(g_row[0:1, 0:1], f"g{j}")
            gb = bcast(g_row, "gb")

            # pre = t + g * (a2o | o2a); LN over H; blend into tok in place
            src = mix_src["x"]
            pre = big.tile([P, HT, BS], bf16, tag="pre", name="pre")
            _st2 = pps.tile([33, BS], f32, tag="st", name="stat2")
            stat2 = (_st2[0:1, :], _st2[32:33, :])
            pairs2 = []
            for kt in range(HT):
                tmp = wk.tile([P, BS], bf16, tag="hm")
                nc.vector.tensor_mul(tmp, gb, src[:, kt, :])
                nc.vector.tensor_add(pre[:, kt, :], tmp, tk(t_m, kt))
                sq = sqp.tile([P, BS], bf16, tag="sq")
                nc.scalar.activation(sq, pre[:, kt, :], AF.Square)
                pairs2.append((pre[:, kt, :], sq))
            colsum_stats(stat2, pairs2)
            negmu2, rinv2 = ln_rows(stat2, H, "u")
            mb = bcast(negmu2, "mb")
            rb2 = bcast(rinv2, "rb2")
            wcol = 16 if is_audio else 0
            bcol = 24 if is_audio else 8
            for kt in range(HT):
                nc.vector.tensor_add(pre[:, kt, :], pre[:, kt, :], mb)
                nc.vector.tensor_mul(pre[:, kt, :], pre[:, kt, :], rb2)
                nc.vector.tensor_scalar(
                    pre[:, kt, :], pre[:, kt, :],
                    lnv[:, wcol + kt : wcol + kt + 1], lnv[:, bcol + kt : bcol + kt + 1],
                    op0=OP.mult, op1=OP.add,
                )
                # blend = big_mask*t + small_mask*(upd - t), in place into tok
                bm = ma_t if is_audio else mo_t[j]
                sm = am_t if is_audio else pv_t[j]
                d2 = wk.tile([P, BS], bf16, tag="d2", name="d2")
                if kt % 2 == 0:
                    pe_warm(pre[0:1, kt, 0:1], f"bl{j}_{kt}")
                nc.vector.tensor_sub(d2, pre[:, kt, :], tk(t_m, kt))
                nc.vector.tensor_mul(d2, d2, sm)
                nc.vector.tensor_mul(tk(t_m, kt), tk(t_m, kt), bm)
                nc.vector.tensor_add(tk(t_m, kt), tk(t_m, kt), d2)
                if not is_audio:
                    if j == 0:
                        nc.vector.tensor_mul(omt[:, kt, :], cm_t[j], tk(mj, kt))
                    else:
                        tmp2 = wk.tile([P, BS], bf16, tag="hm")
                        nc.vector.tensor_mul(tmp2, cm_t[j], tk(mj, kt))
                        nc.vector.tensor_add(omt[:, kt, :], omt[:, kt, :], tmp2)

        for j, mj in enumerate(OTHERS):
            gate_and_mix(j, mj)

        # ---- o2a = others_mean @ o2a_w.T ----
        o2ar = big.tile([P, HT, BS], bf16, tag="axr")
        for ho in range(HT):
            wt = wgp.tile([P, HT, P], bf16, tag="ww")
            nc.sync.dma_start(out=wt, in_=O2AT.ap()[ho])
            ps = ppz.tile([P, BS], f32, tag="z")
            for kt in range(HT):
                nc.tensor.matmul(ps, wt[:, kt, :], omt[:, kt, :], start=(kt == 0), stop=(kt == HT - 1))
            nc.scalar.activation(o2ar[:, ho, :], ps, AF.Copy, bias=0.0, scale=1.0)
        mix_src["x"] = o2ar

        gate_and_mix(3, AUDIO)

        # ---- LMF: z per (ht,r,m); prod over m on DVE; rank-sum on PE ----
        accs = big.tile([P, HT, BS], bf16, tag="accs")
        for ht in range(HT):
            acc_ps = ppa.tile([P, BS], f32, tag="acc", name=f"acc{ht}")
            for r in range(R):
                zps = []
                s01 = None
                z0c = None
                ftm = ftp.tile([P, M, 9, P], bf16, tag="ft")
                nc.sync.dma_start(out=ftm, in_=FT.ap()[ht, r])
                for m in range(M):
                    zp = ppz.tile([P, BS], f32, tag="z")
                    # tail first (full width): bias*mask + 1*(1-mask); main
                    # matmuls then accumulate only over active column spans.
                    sp = spans[m]
                    nc.tensor.matmul(
                        zp, ftm[0:2, m, 8, :], uvt[m], start=True, stop=(len(sp) == 0)
                    )
                    for kt in range(HT):
                        for si, (a, b) in enumerate(sp):
                            inst = nc.tensor.matmul(
                                zp[:, a:b], ftm[:, m, kt, :], tok[:, m, kt, a:b],
                                start=False,
                                stop=(kt == HT - 1 and si == len(sp) - 1),
                            )
                            if si > 0:
                                # same stationary as the previous span matmul —
                                # skip the redundant LDWEIGHTS
                                inst.ins.ldweights = False
                    zps.append(zp)
                    # PSUM has one DVE read port: stage one factor of each
                    # pair through SBUF via the (otherwise idle) ScalarE.
                    if m == 0:
                        z0c = wk.tile([P, BS], bf16, tag="s01", name="z0c")
                        nc.scalar.activation(z0c, zps[0], AF.Copy, bias=0.0, scale=1.0)
                    if m == 1:
                        s01 = wk.tile([P, BS], bf16, tag="s01", name="s01")
                        nc.vector.tensor_mul(s01, z0c, zps[1])
                    if m == 2:
                        z0c = wk.tile([P, BS], bf16, tag="s23", name="z2c")
                        nc.scalar.activation(z0c, zps[2], AF.Copy, bias=0.0, scale=1.0)
                s23 = wk.tile([P, BS], bf16, tag="s23", name="s23")
                nc.vector.tensor_mul(s23, z0c, zps[3])
                prod = wk.tile([P, BS], bf16, tag="prod")
                nc.vector.tensor_mul(prod, s01, s23)
                nc.tensor.matmul(acc_ps, eye, prod, start=(r == 0), stop=(r == R - 1))
            # accs = acc + lmf_bias (bias col folded into the copy)
            nc.scalar.activation(
                accs[:, ht, :], acc_ps, AF.Identity,
                bias=lnv[:, 72 + ht : 72 + ht + 1], scale=1.0,
            )

        # ---- output MLP ----
        _st3 = pps.tile([33, BS], f32, tag="st", name="stat3")
        stat3 = (_st3[0:1, :], _st3[32:33, :])
        pairs3 = []
        for kt in range(HT):
            sq = sqp.tile([P, BS], bf16, tag="sq")
            nc.scalar.activation(sq, accs[:, kt, :], AF.Square)
            pairs3.append((accs[:, kt, :], sq))
        colsum_stats(stat3, pairs3)
        negmu3, rinv3 = ln_rows(stat3, H, "l1")
        mb3 = bcast(negmu3, "mb")
        rb3 = bcast(rinv3, "rb2")
        for kt in range(HT):
            nc.vector.tensor_add(accs[:, kt, :], accs[:, kt, :], mb3)
            nc.vector.tensor_mul(accs[:, kt, :], accs[:, kt, :], rb3)
            nc.vector.tensor_scalar(
                accs[:, kt, :], accs[:, kt, :],
                lnv[:, 32 + kt : 32 + kt + 1], lnv[:, 40 + kt : 40 + kt + 1],
                op0=OP.mult, op1=OP.add,
            )

        # h2 = gelu(h1 @ out_w.T + out_b); LN2; write out
        h2 = big.tile([P, HT, BS], bf16, tag="abs")
        _st4 = pps.tile([33, BS], f32, tag="st", name="stat4")
        stat4 = (_st4[0:1, :], _st4[32:33, :])
        pairs4 = []
        for ho in range(HT):
            wt = wgp.tile([P, HT, P], bf16, tag="ww")
            nc.sync.dma_start(out=wt, in_=OUTWT.ap()[ho])
            ps = ppz.tile([P, BS], f32, tag="z")
            for kt in range(HT):
                nc.tensor.matmul(ps, wt[:, kt, :], accs[:, kt, :], start=(kt == 0), stop=(kt == HT - 1))
            nc.scalar.activation(
                h2[:, ho, :], ps, AF.Gelu, bias=lnv[:, 64 + ho : 64 + ho + 1], scale=1.0
            )
            sq = sqp.tile([P, BS], bf16, tag="sq")
            nc.scalar.activation(sq, h2[:, ho, :], AF.Square)
            pairs4.append((h2[:, ho, :], sq))
        colsum_stats(stat4, pairs4)
        negmu4, rinv4 = ln_rows(stat4, H, "l2")
        mb4 = bcast(negmu4, "mb")
        rb4 = bcast(rinv4, "rb2")
        for kt in range(HT):
            fin = wk.tile([P, BS], f32, tag="fin")
            nc.vector.tensor_add(fin, h2[:, kt, :], mb4)
            nc.vector.tensor_mul(fin, fin, rb4)
            nc.vector.tensor_scalar(
                fin, fin, lnv[:, 48 + kt : 48 + kt + 1], lnv[:, 56 + kt : 56 + kt + 1],
                op0=OP.mult, op1=OP.add,
            )
            nc.sync.dma_start(out=OUT.ap()[kt * P : (kt + 1) * P, :], in_=fin)

    nc.compile()
    return nc


def _host_prep(inputs, perms):
    tokens = np.asarray(inputs["tokens"], np.float32)
    token_mask = np.asarray(inputs["token_mask"])
    mask_f = token_mask.astype(np.float32)

    mo = mask_f[:, list(OTHERS)]                      # [B,3]
    ma = mask_f[:, AUDIO]                             # [B]
    pv = mo * ma[:, None]                             # [B,3]
    winv = (1.0 / np.clip(mo.sum(1), 1.0, None)).astype(np.float32)
    aum = ma * (mo.max(1) > 0)                        # [B]

    go_w1 = np.asarray(inputs["go_w1"], np.float32)
    ga_w1 = np.asarray(inputs["ga_w1"], np.float32)

    def gate_prep(w1, b1, lnw, lnb):
        W1w = w1 * lnw[None, :]                       # [MID, 3H]
        c1 = np.ascontiguousarray(W1w.sum(1).reshape(1, MID))
        cb = w1 @ lnb + b1                            # [MID]
        Wblocks = np.ascontiguousarray(W1w.T).reshape(3 * HT, P, MID)
        return Wblocks, c1, cb

    WGOv, c1go, cbgo = gate_prep(
        go_w1, np.asarray(inputs["go_b1"], np.float32),
        np.asarray(inputs["ln_go_w"], np.float32), np.asarray(inputs["ln_go_b"], np.float32),
    )
    WGAv, c1ga, cbga = gate_prep(
        ga_w1, np.asarray(inputs["ga_b1"], np.float32),
        np.asarray(inputs["ln_ga_w"], np.float32), np.asarray(inputs["ln_ga_b"], np.float32),
    )
    CBv = np.ascontiguousarray(
        np.concatenate([cbgo.reshape(MT, P).T, cbga.reshape(MT, P).T], axis=1)
    ).astype(np.float32)                              # [P, 8]
    W2v = np.stack(
        [np.asarray(inputs["go_w2"], np.float32).reshape(MID),
         np.asarray(inputs["ga_w2"], np.float32).reshape(MID)], axis=1
    )                                                 # [MID, 2]
    W2v = np.ascontiguousarray(W2v.reshape(MT, P, 2).transpose(1, 0, 2))
    SCv = np.zeros((1, 8), np.float32)
    SCv[0, 0] = np.asarray(inputs["go_b2"], np.float32).reshape(-1)[0]
    SCv[0, 1] = np.asarray(inputs["ga_b2"], np.float32).reshape(-1)[0]
    SCv[0, 2] = EPS

    def tile_blocks(w):
        wt = np.ascontiguousarray(np.asarray(w, np.float32).T)    # [H_in, H_out]
        # [ho, p, kt, c] = wT[kt*P+p, ho*P+c]
        return np.ascontiguousarray(
            wt.reshape(HT, P, HT, P).transpose(2, 1, 0, 3)
        ).astype(nbf)

    A2OTv = tile_blocks(inputs["a2o_w"])
    O2ATv = tile_blocks(inputs["o2a_w"])
    OUTWTv = tile_blocks(inputs["out_w"])

    def cols(name):
        return np.asarray(inputs[name], np.float32).reshape(HT, P).T

    LNVv = np.zeros((P, 80), np.float32)
    for i, name in enumerate(
        ["ln_o_w", "ln_o_b", "ln_a_w", "ln_a_b", "out_ln1_w", "out_ln1_b",
         "out_ln2_w", "out_ln2_b", "out_b", "lmf_bias"]
    ):
        LNVv[:, 8 * i : 8 * (i + 1)] = cols(name)

    factors = np.asarray(inputs["factors"], np.float32)
    rank_w = np.asarray(inputs["rank_w"], np.float32)
    Ff = factors.copy()
    Ff[AUDIO] = Ff[AUDIO] * rank_w[:, None, None]
    FTv = np.zeros((HT, R, P, M, 9, P), np.float32)
    # main [m, r, kt, pk, ht, ph] -> [ht, r, pk, m, kt, ph]
    main = Ff[:, :, 1:, :].reshape(M, R, HT, P, HT, P)
    FTv[:, :, :, :, :8, :] = main.transpose(4, 1, 3, 0, 2, 5)
    bias = Ff[:, :, 0, :].reshape(M, R, HT, P)           # [m, r, ht, ph]
    FTv[:, :, 0, :, 8, :] = bias.transpose(2, 1, 0, 3)
    ones_row = np.ones((HT, R, M, P), np.float32)
    ones_row[:, :, AUDIO, :] = rank_w[None, :, None]
    FTv[:, :, 1, :, 8, :] = ones_row
    FTv = FTv.astype(nbf)

    shared = dict(
        WGO=WGOv.astype(nbf), WGA=WGAv.astype(nbf),
        WGOe=c1go.astype(nbf), WGAe=c1ga.astype(nbf),
        W2=W2v.astype(nbf), CB=CBv, SC=SCv,
        A2OT=A2OTv, O2AT=O2ATv, OUTWT=OUTWTv, LNV=LNVv, FT=FTv,
        EYE=np.eye(P, dtype=nbf),
    )

    in_maps = []
    for c in range(NCORES):
        sl = perms[c]
        tokTv = np.ascontiguousarray(
            tokens[sl].transpose(1, 2, 0)
        ).astype(nbf)                                  # [M, H, BS]
        mv = np.zeros((8, BS), np.float32)
        mv[0:3] = (pv[sl].T > 0)
        mv[3:6] = (mo[sl].T > 0)
        mv[6] = aum[sl] > 0
        mv[7] = ma[sl] > 0
        cmv = np.ascontiguousarray((mo[sl] * winv[sl, None]).T)
        uvv = np.zeros((M, 2, BS), np.float32)
        uvv[:, 0, :] = mask_f[sl].T
        uvv[:, 1, :] = 1.0 - mask_f[sl].T
        in_maps.append(dict(
            tokT=tokTv, mrows=mv.astype(nbf), cmrows=cmv.astype(nbf),
            uv=uvv.astype(nbf), **shared,
        ))
    return in_maps


def _pack_plan(token_mask):
    """Sort batches by mask pattern into per-core columns with identical
    per-pattern quotas across cores (so one compiled NEFF serves all 8),
    plus per-modality active-column spans shared by all cores."""
    mask = np.asarray(token_mask).astype(np.int64)
    code = (mask[:, 0] | (mask[:, 1] << 1) | (mask[:, 2] << 2) | (mask[:, 3] << 3))
    idx_by_p = {p: np.nonzero(code == p)[0] for p in range(16)}
    q = {p: len(idx_by_p[p]) // NCORES for p in range(16)}
    S = (B - NCORES * sum(q.values())) // NCORES  # overflow slots per core
    overflow = np.concatenate(
        [idx_by_p[p][NCORES * q[p]:] for p in PORDER]
    ) if S else np.zeros((0,), np.int64)
    perms = []
    for c in range(NCORES):
        parts = [idx_by_p[p][c * q[p]:(c + 1) * q[p]] for p in PORDER]
        if S:
            parts.append(overflow[c * S:(c + 1) * S])
        perms.append(np.concatenate(parts).astype(np.int64))
        assert perms[-1].shape == (BS,)
    # segment layout (identical across cores): PORDER segments + overflow
    segs = [(p, q[p]) for p in PORDER] + [(-1, S)]  # -1: overflow = all active
    spans = {}
    for m in range(M):
        sp, off = [], 0
        cur = None
        for p, ln in segs:
            act = True if p == -1 else bool((p >> m) & 1)
            if ln:
                if act:
                    if cur is None:
                        cur = [off, off + ln]
                    else:
                        cur[1] = off + ln
                elif cur is not None:
                    sp.append(tuple(cur)); cur = None
            off += ln
        if cur is not None:
            sp.append(tuple(cur))
        spans[m] = tuple(sp)
    return perms, tuple(sorted(spans.items()))


def kernel(**inputs):
    global LAST_RESULTS
    perms, spans_t = _pack_plan(inputs["token_mask"])
    if spans_t not in _cached:
        _cached[spans_t] = _build(dict(spans_t))
    in_maps = _host_prep(inputs, perms)
    res = run_bass_kernel_spmd(
        _cached[spans_t], in_maps, core_ids=list(range(NCORES)), trace=TRACE
    )
    LAST_RESULTS = res
    out = np.zeros((B, H), np.float32)
    for c in range(NCORES):
        out[perms[c]] = res.results[c]["outT"].T
    return np.ascontiguousarray(out)
